# revision 26
# baseline (speedup 1.0000x reference)
"""Trainium2 Bass kernel for nn_ARPredVideoVanilla (8-core data-parallel).

Strategy: pure data parallelism over batch (B=8 -> 1 batch element per core,
no collectives).  Activations live feature-major in SBUF: (128 partitions =
feature chunk, KO feature tiles, tokens free).  Weights are pre-folded on the
host (LN scale/bias folded into the following matmul, attention scale folded
into W_q, K-bias dropped (softmax row-shift invariant), V-bias folded into the
output-projection bias) and shipped as bf16.  The stochastic block mask is
evaluated on the host and compiled into the graph: blocked (query,key) frame
blocks are simply never computed.  Softmax runs without max-subtraction
(scores are O(1) by construction); exp runs on ScalarE with accum_out row sums.
"""

import sys

sys.path.insert(0, "/opt/trn_rl_repo")

import numpy as np
import ml_dtypes

BF16 = ml_dtypes.bfloat16

# ---- model dims (hardcoded from the problem spec) ----
B, T, V = 8, 4, 3
C = V * T                      # 12
H, W, PH, PW = 128, 256, 16, 16
HP, WP = H // PH, W // PW      # 8, 16
P = HP * WP                    # 128 patches/frame
S = T * P                      # 512
D, NH, HD, L = 768, 12, 64, 8
DD, NHD, HDD, LD = 512, 8, 64, 4
MLP, MLPD = 3072, 2048
OUT = PH * PW * V              # 768
MASK_RATIO = 0.8
EPS = 1e-5
PP = 128  # partitions


# ------------------------------------------------------------------
# host-side preparation: fold biases/scales, transpose, cast to bf16
# ------------------------------------------------------------------

def _prep(inputs):
    f32 = np.float32
    g = {k: np.asarray(v, dtype=f32) for k, v in inputs.items()}

    w = {}

    def bf(a):
        return np.ascontiguousarray(a.astype(BF16))

    def pcol(bias):  # (M,) -> (128, M//128) per-partition layout, tile-major
        M = bias.shape[0]
        return np.ascontiguousarray(bias.reshape(M // PP, PP).T.astype(f32))

    # patch data, per core: x[b] (T,C,H,W) -> xfT (C*PH*PW, T*P)
    x = g["x"]  # (B,T,C,H,W)
    xf = x.reshape(B, T, C, HP, PH, WP, PW).transpose(0, 1, 3, 5, 2, 4, 6)
    xf = xf.reshape(B, T * P, C * PH * PW)          # (B, 512, 3072)
    xfT = np.ascontiguousarray(np.swapaxes(xf, 1, 2))  # (B, 3072, 512)
    xfT_bf = [bf(xfT[b]) for b in range(B)]

    # conv: wk (3072, 768); pos_eff (768, 512) f32 with conv_b folded in
    wk = g["conv_w"].reshape(D, C * PH * PW).T      # (3072, 768)
    w["wk"] = bf(wk)
    pos = g["pos_emb"][0].T + g["conv_b"][:, None]  # (768, 512)
    w["pos"] = np.ascontiguousarray(pos.astype(f32))

    scale = HD ** -0.5
    for l in range(L):
        s1, b1 = g["enc_ln1_s"][l], g["enc_ln1_b"][l]
        Wqkv = g["enc_qkv_w"][l]                    # (768, 2304)
        Wq_eff = (s1[:, None] * Wqkv).copy()
        Wq_eff[:, :D] *= scale
        w[f"eqk{l}"] = bf(Wq_eff[:, :2 * D])
        w[f"evw{l}"] = bf(Wq_eff[:, 2 * D:])
        cb = b1 @ Wqkv                              # folded LN bias through qkv
        w[f"eqb{l}"] = pcol(cb[:D] * scale)         # q bias (per-partition)
        # k bias dropped (softmax row-invariant); v bias folded into proj bias
        w[f"eproj{l}"] = bf(g["enc_proj_w"][l])
        pb = g["enc_proj_b"][l] + cb[2 * D:] @ g["enc_proj_w"][l]
        w[f"epb{l}"] = pcol(pb)
        s2, b2 = g["enc_ln2_s"][l], g["enc_ln2_b"][l]
        W1 = g["enc_mlp_w1"][l]
        w[f"em1{l}"] = bf(s2[:, None] * W1)
        w[f"em1b{l}"] = pcol(b2 @ W1 + g["enc_mlp_b1"][l])
        w[f"em2{l}"] = bf(g["enc_mlp_w2"][l])
        w[f"em2b{l}"] = pcol(g["enc_mlp_b2"][l])

    w["e2dw"] = bf(g["e2d_w"])                      # (768, 512)
    w["e2db"] = pcol(g["e2d_b"])
    w["dq"] = np.ascontiguousarray(g["dec_query"][0].T.astype(f32))  # (512,128)

    dscale = HDD ** -0.5
    for l in range(LD):
        s1, b1 = g["dec_ln1_s"][l], g["dec_ln1_b"][l]
        Wq = g["dec_qkv_w"][l, 0]
        w[f"dwq{l}"] = bf(s1[:, None] * Wq * dscale)
        w[f"dqb{l}"] = pcol((b1 @ Wq + g["dec_qkv_b"][l, 0]) * dscale)
        w[f"dwk{l}"] = bf(g["dec_qkv_w"][l, 1])     # k bias dropped
        w[f"dwv{l}"] = bf(g["dec_qkv_w"][l, 2])
        w[f"dwo{l}"] = bf(g["dec_out_w"][l])
        ob = g["dec_out_b"][l] + g["dec_qkv_b"][l, 2] @ g["dec_out_w"][l]
        w[f"dob{l}"] = pcol(ob)
        s2, b2 = g["dec_ln2_s"][l], g["dec_ln2_b"][l]
        W1 = g["dec_mlp_w1"][l]
        w[f"dm1{l}"] = bf(s2[:, None] * W1)
        w[f"dm1b{l}"] = pcol(b2 @ W1 + g["dec_mlp_b1"][l])
        w[f"dm2{l}"] = bf(g["dec_mlp_w2"][l])
        w[f"dm2b{l}"] = pcol(g["dec_mlp_b2"][l])

    sh, bh = g["head_ln_s"], g["head_ln_b"]
    w["hw"] = bf(sh[:, None] * g["head_w"])         # (512, 768)
    hb = bh @ g["head_w"] + g["head_b"]             # (768,) per-FREE bias
    w["hbb"] = np.ascontiguousarray(
        np.broadcast_to(hb[None, :], (PP, OUT)).astype(f32))

    w["ident"] = np.ascontiguousarray(np.eye(PP, dtype=np.float32).astype(BF16))

    # block mask: allowed[l][qi] = tuple of allowed key-frame blocks
    mr = g["mask_rand"]                             # (L, T, T)
    allowed = []
    for l in range(L):
        per_q = []
        for i in range(T):
            ks = [j for j in range(T)
                  if j <= i or not (mr[l, i, j] < MASK_RATIO)]
            per_q.append(tuple(ks))
        allowed.append(per_q)

    return w, xfT_bf, allowed


# ------------------------------------------------------------------
# Tile tail-drain patch: this walrus build rejects >1 sync wait per
# instruction at the kernel-tail drain; split the waits across NOPs.
# ------------------------------------------------------------------

def _patch_tile():
    import concourse.tile as tile
    from concourse.vector_clock import ScopedClock, VectorClock

    if getattr(tile.TileContext, "_drain_patched", False):
        return

    def _drain_and_barrier_chunked(self, tick_clock, wait_clock):
        g = list(tick_clock.global_clock)
        procs = [i for i, v in enumerate(g) if v > 0]
        for p in procs:
            sub = [0] * len(g)
            sub[p] = g[p]
            nop_inst = self.nc.sync.nop(nofuse=True)
            wait_clock.add_sem_waits(
                nop_inst.ins, ScopedClock({None: VectorClock(sub)}))
        self.nc.sync.drain()
        self.nc.all_engine_barrier()
        assert self.sems is not None
        popped = self.nc._tile_sem_poison_stack.pop()
        assert popped is self._sem_poison
        self.nc.clear_and_free_semaphores(list(self.sems.allocated().values()))
        self.nc.all_engine_barrier()

    tile.TileContext._drain_and_barrier = _drain_and_barrier_chunked

    # This walrus build also rejects >1 sync wait on regular engine
    # instructions (Matmult etc.).  Hoist excess waits onto same-engine
    # NOPs inserted immediately before the instruction.
    from concourse import mybir as _mybir

    _orig_lower = tile.TileContext._lower_ordered_insts

    def _split_waits_and_lower(self, ordered):
        nctr = [0]
        for bb_name, insts in ordered.items():
            new_list = []
            for inst in insts:
                si = getattr(inst, "sync_info", None)
                waits = list(si.on_wait) if si is not None else []
                if len(waits) > 1:
                    imm = [w for w in waits if w.wait_reg is None]
                    reg = [w for w in waits if w.wait_reg is not None]
                    keep = imm[:1] + reg  # keep one imm (plus any reg waits)
                    excess = imm[1:]
                    for w in excess:
                        nctr[0] += 1
                        nop = _mybir.InstNoOp(
                            name=f"{inst.name}-wsplit{nctr[0]}", ins=[], outs=[])
                        nop.engine = inst.engine
                        nop.sync_info = _mybir.SyncInfo(
                            on_wait=[w], on_update=[])
                        self.nc.register_instruction(nop, overwrite=True)
                        new_list.append(nop)
                    inst.sync_info = _mybir.SyncInfo(
                        on_wait=keep, on_update=list(si.on_update))
                new_list.append(inst)
            insts[:] = new_list
        return _orig_lower(self, ordered)

    tile.TileContext._lower_ordered_insts = _split_waits_and_lower
    tile.TileContext._drain_patched = True


# ------------------------------------------------------------------
# graph builder
# ------------------------------------------------------------------

def _build(allowed, dbg=()):
    import concourse.bass as bass
    import concourse.tile as tile
    from concourse import mybir

    _patch_tile()
    f32 = mybir.dt.float32
    bf16 = mybir.dt.bfloat16
    AF = mybir.ActivationFunctionType
    OP = mybir.AluOpType

    nc = bass.Bass()

    # ---- DRAM parameters ----
    dp = {}

    def din(name, shape, dtype):
        dp[name] = nc.declare_dram_parameter(name, list(shape), dtype, isOutput=False)
        return dp[name]

    din("xfT", (24 * PP, S), bf16)
    din("wk", (24 * PP, D), bf16)
    din("pos", (D, S), f32)
    for l in range(L):
        din(f"eqk{l}", (D, 2 * D), bf16)
        din(f"evw{l}", (D, D), bf16)
        din(f"eqb{l}", (PP, 6), f32)
        din(f"eproj{l}", (D, D), bf16)
        din(f"epb{l}", (PP, 6), f32)
        din(f"em1{l}", (D, MLP), bf16)
        din(f"em1b{l}", (PP, 24), f32)
        din(f"em2{l}", (MLP, D), bf16)
        din(f"em2b{l}", (PP, 6), f32)
    din("e2dw", (D, DD), bf16)
    din("e2db", (PP, 4), f32)
    din("dq", (DD, P), f32)
    for l in range(LD):
        din(f"dwq{l}", (DD, DD), bf16)
        din(f"dqb{l}", (PP, 4), f32)
        din(f"dwk{l}", (DD, DD), bf16)
        din(f"dwv{l}", (DD, DD), bf16)
        din(f"dwo{l}", (DD, DD), bf16)
        din(f"dob{l}", (PP, 4), f32)
        din(f"dm1{l}", (DD, MLPD), bf16)
        din(f"dm1b{l}", (PP, 16), f32)
        din(f"dm2{l}", (MLPD, DD), bf16)
        din(f"dm2b{l}", (PP, 4), f32)
    din("hw", (DD, OUT), bf16)
    din("ident", (PP, PP), bf16)
    din("hbb", (PP, OUT), f32)
    out_ext = nc.declare_dram_parameter("out", [P, OUT], f32, isOutput=True)
    dbg_ext = {name: nc.declare_dram_parameter(name, [PP, 6, S], f32, isOutput=True)
               for name in dbg}

    with tile.TileContext(nc) as tc:
        with (
            tc.tile_pool(name="consts", bufs=1) as consts,
            tc.tile_pool(name="wpool", bufs=4) as wpool,
            tc.tile_pool(name="bias", bufs=6) as biasp,
            tc.tile_pool(name="act", bufs=1) as act,
            tc.tile_pool(name="tmp", bufs=2) as tmp,
            tc.tile_pool(name="hsqp", bufs=1) as hsqp,
            tc.tile_pool(name="bigp", bufs=1) as bigp,
            tc.tile_pool(name="attn", bufs=10) as attnp,
            tc.tile_pool(name="rrsp", bufs=2) as rrsp,
            tc.tile_pool(name="small", bufs=2) as small,
            tc.tile_pool(name="tiny", bufs=8) as tiny,
            tc.tile_pool(name="pp_mm", bufs=2, space="PSUM") as pp_mm,
            tc.tile_pool(name="pp_sc", bufs=2, space="PSUM") as pp_sc,
            tc.tile_pool(name="pp_pv", bufs=1, space="PSUM") as pp_pv,
            tc.tile_pool(name="pp_st", bufs=2, space="PSUM") as pp_st,
            tc.tile_pool(name="pp_bc", bufs=1, space="PSUM") as pp_bc,
        ):
            ones_f32 = consts.tile([PP, 1], f32)
            nc.vector.memset(ones_f32, 1.0)
            ones_row = consts.tile([1, PP], f32)
            nc.vector.memset(ones_row, 1.0)
            ones_bf16 = consts.tile([PP, 1], bf16)
            nc.vector.memset(ones_bf16, 1.0)
            ones_row_bf = consts.tile([1, PP], bf16)
            nc.vector.memset(ones_row_bf, 1.0)
            eps_t = consts.tile([1, 1], f32)
            nc.vector.memset(eps_t, EPS)

            def load_w(name, KO, M, dtype=bf16, tag="w"):
                t = wpool.tile([PP, KO, M], dtype, tag=tag)
                nc.sync.dma_start(
                    out=t, in_=dp[name][:].rearrange("(ko p) m -> p ko m", p=PP))
                return t

            W_SLOT = 6144  # bf16 elems per partition in a weight slot

            def load_b(name, MO):
                t = biasp.tile([PP, MO], f32, tag="b")
                nc.sync.dma_start(out=t, in_=dp[name][:])
                return t

            # dense matmul with chunked weight streaming from DRAM.
            # out feature-major; rhs (128, KO, N); evac(m, psum)
            def dense(wname, KO, MO, rhs_sb, N, evac):
                M = MO * PP
                mch_cols = max(PP, (W_SLOT // KO) // PP * PP)
                wap = dp[wname][:].rearrange("(ko p) m -> p ko m", p=PP)
                for c0 in range(0, M, mch_cols):
                    mch = min(mch_cols, M - c0)
                    wt = wpool.tile([PP, KO, mch], bf16, tag="w")
                    nc.sync.dma_start(out=wt, in_=wap[:, :, c0:c0 + mch])
                    for mi in range(mch // PP):
                        m = c0 // PP + mi
                        ps = pp_mm.tile([PP, 512], f32, tag="mm")
                        for k in range(KO):
                            nc.tensor.matmul(
                                ps[:, :N],
                                lhsT=wt[:, k, mi * PP:(mi + 1) * PP],
                                rhs=rhs_sb[:, k, :],
                                start=(k == 0), stop=(k == KO - 1))
                        evac(m, ps[:, :N])

            # layer norm, feature-major input (128, KO, W) f32 -> bf16 out
            def lnorm(h_sb, KO, Wd, y_sb):
                Dm = KO * PP
                hb = hsqp.tile([PP, KO, Wd], bf16, tag="hb")
                hsq = hsqp.tile([PP, KO, Wd], bf16, tag="hsq")
                st = pp_st.tile([33, Wd], f32, tag="st")
                for k in range(KO):
                    nc.vector.tensor_copy(hb[:, k, :], h_sb[:, k, :])
                    nc.tensor.matmul(st[0:1, :], lhsT=ones_bf16, rhs=hb[:, k, :],
                                     start=(k == 0), stop=(k == KO - 1))
                    nc.scalar.activation(hsq[:, k, :], h_sb[:, k, :], AF.Square)
                    nc.tensor.matmul(st[32:33, :], lhsT=ones_bf16, rhs=hsq[:, k, :],
                                     start=(k == 0), stop=(k == KO - 1))
                mean = small.tile([1, Wd], f32, tag="s1")
                nc.vector.tensor_scalar_mul(mean, st[0:1, :], 1.0 / Dm)
                var = small.tile([1, Wd], f32, tag="s2")
                nc.vector.tensor_scalar_mul(var, st[32:33, :], 1.0 / Dm)
                msq = small.tile([1, Wd], f32, tag="s3")
                nc.vector.tensor_mul(msq, mean, mean)
                nc.vector.tensor_sub(var, var, msq)
                # 1/sqrt(var+eps) = exp(-0.5*ln(var+eps)): stays in the
                # exp/ln ACT table set (no sqrt-set switch, no slow DVE
                # reciprocal)
                nc.scalar.activation(var, var, AF.Ln, bias=eps_t)
                inv = small.tile([1, Wd], f32, tag="s5")
                nc.scalar.activation(inv, var, AF.Exp, scale=-0.5)
                mean_bf = small.tile([1, Wd], bf16, tag="s6")
                nc.vector.tensor_copy(mean_bf, mean)
                inv_bf = small.tile([1, Wd], bf16, tag="s7")
                nc.vector.tensor_copy(inv_bf, inv)
                mb = pp_bc.tile([PP, Wd], f32, tag="bc")
                nc.tensor.matmul(mb, lhsT=ones_row_bf, rhs=mean_bf,
                                 start=True, stop=True)
                for k in range(KO):
                    nc.vector.tensor_tensor(
                        y_sb[:, k, :], h_sb[:, k, :], mb, OP.subtract)
                ib = pp_bc.tile([PP, Wd], f32, tag="bc")
                nc.tensor.matmul(ib, lhsT=ones_row_bf, rhs=inv_bf,
                                 start=True, stop=True)
                for k in range(KO):
                    nc.vector.tensor_tensor(
                        y_sb[:, k, :], y_sb[:, k, :], ib, OP.mult)

            # attention, transposed-scores formulation: no p transposes.
            # The two heads of a 128-partition pair are interleaved matmul-by-
            # matmul so they land on disjoint PE row/col groups and execute
            # concurrently.  Row sums for both heads share one PSUM tile at
            # partitions 0 and 32.
            def attention(q_sb, k_sb, vT_sb, o_sb, n_heads, n_q_tiles,
                          allowed_per_qi):
                kj_all = sorted({kj for qi in range(n_q_tiles)
                                 for kj in allowed_per_qi[qi]})
                kj_to_qi = {kj: [qi for qi in range(n_q_tiles)
                                 if kj in allowed_per_qi[qi]] for kj in kj_all}

                def qi_runs(qis):
                    runs = []
                    i = 0
                    while i < len(qis):
                        j = i
                        while j + 1 < len(qis) and qis[j + 1] == qis[j] + 1:
                            j += 1
                        runs.append(qis[i:j + 1])
                        i = j + 1
                    return runs

                W0 = n_q_tiles * PP
                # pack consecutive key blocks into <=512-col score tiles so
                # each ScalarE exp covers more columns (fixed ~352cyc op
                # overhead amortizes)
                packs = []
                cur = []
                cur_cols = 0
                for kj in kj_all:
                    nc_kj = len(kj_to_qi[kj]) * PP
                    if cur and cur_cols + nc_kj > 512:
                        packs.append(cur)
                        cur, cur_cols = [], 0
                    cur.append(kj)
                    cur_cols += nc_kj
                if cur:
                    packs.append(cur)
                for pair in range(n_heads // 2):
                    pt2 = [{}, {}]
                    # scores + exp, subs interleaved per key block
                    for pack in packs:
                        pcols = sum(len(kj_to_qi[kj]) for kj in pack) * PP
                        sc2 = []
                        for sub in range(2):
                            sc2.append(pp_sc.tile([PP, 512], f32, tag="sc",
                                                  name=f"sc{sub}"))
                        base = 0
                        for kj in pack:
                            qis = kj_to_qi[kj]
                            for run in qi_runs(qis):
                                col = base + qis.index(run[0]) * PP
                                for sub in range(2):
                                    b0 = 64 * sub
                                    nc.tensor.matmul(
                                        sc2[sub][:, col:col + len(run) * PP],
                                        lhsT=k_sb[b0:b0 + 64, pair,
                                                  kj * PP:(kj + 1) * PP],
                                        rhs=q_sb[b0:b0 + 64, pair,
                                                 run[0] * PP:
                                                 (run[-1] + 1) * PP],
                                        start=True, stop=True)
                            base += len(qis) * PP
                        for sub in range(2):
                            pt = attnp.tile([PP, 512], bf16, tag="p")
                            nc.scalar.activation(pt[:, :pcols],
                                                 sc2[sub][:, :pcols], AF.Exp)
                            base = 0
                            for kj in pack:
                                qis = kj_to_qi[kj]
                                pt2[sub][kj] = (pt, {qi: base + i * PP
                                                     for i, qi
                                                     in enumerate(qis)})
                                base += len(qis) * PP
                    # row sums: separate tiles per sub (same-bank PE-write +
                    # ACT-read on disjoint partitions is a HW fault)
                    rsps2 = [pp_st.tile([1, 512], f32, tag="st", name="rs0"),
                             pp_st.tile([1, 512], f32, tag="st", name="rs1")]
                    seen = [[0] * n_q_tiles, [0] * n_q_tiles]
                    nkj = {qi: len(allowed_per_qi[qi])
                           for qi in range(n_q_tiles)}
                    for kj in kj_all:
                        qis = kj_to_qi[kj]
                        for run in qi_runs(qis):
                            for sub in range(2):
                                pt, cols = pt2[sub][kj]
                                nc.tensor.matmul(
                                    rsps2[sub][0:1,
                                               run[0] * PP:(run[-1] + 1) * PP],
                                    lhsT=ones_bf16,
                                    rhs=pt[:, cols[run[0]]:
                                           cols[run[0]] + len(run) * PP],
                                    start=(seen[sub][run[0]] == 0),
                                    stop=(seen[sub][run[0]]
                                          == nkj[run[0]] - 1))
                            for qi in run:
                                seen[0][qi] += 1
                                seen[1][qi] += 1
                    # 1/rowsum via exp(-ln(x)) on ScalarE, both subs
                    rr2 = []
                    for sub in range(2):
                        rr = small.tile([1, 512], f32, tag="rr", name=f"rrx{sub}")
                        lnr = small.tile([1, 512], f32, tag="s3")
                        nc.scalar.activation(lnr[:, :W0],
                                             rsps2[sub][:, :W0], AF.Ln)
                        nc.scalar.activation(rr[:, :W0], lnr[:, :W0],
                                             AF.Exp, scale=-1.0)
                        rr2.append(rr)
                    rrb = pp_bc.tile([PP, 512], f32, tag="bc")
                    nc.tensor.matmul(rrb[0:64, :W0], lhsT=ones_row[:, :64],
                                     rhs=rr2[0][:, :W0], start=True, stop=True)
                    nc.tensor.matmul(rrb[64:128, :W0], lhsT=ones_row[:, :64],
                                     rhs=rr2[1][:, :W0], start=True, stop=True)
                    rrs = rrsp.tile([PP, 512], bf16, tag="rrs")
                    nc.vector.tensor_copy(rrs[:, :W0], rrb[:, :W0])
                    # PV, subs interleaved, merged over contiguous qi runs
                    po_ps = pp_pv.tile([PP, 512], f32, tag="pv")
                    for kj in kj_all:
                        qis = kj_to_qi[kj]
                        i = 0
                        while i < len(qis):
                            qi0 = qis[i]
                            st0 = (kj == allowed_per_qi[qi0][0])
                            sp0 = (kj == allowed_per_qi[qi0][-1])
                            j = i
                            while (j + 1 < len(qis)
                                   and qis[j + 1] == qis[j] + 1
                                   and (kj == allowed_per_qi[
                                       qis[j + 1]][0]) == st0
                                   and (kj == allowed_per_qi[
                                       qis[j + 1]][-1]) == sp0):
                                j += 1
                            run = qis[i:j + 1]
                            for s2 in range(2):
                                hh = 2 * pair + s2
                                pt, cols = pt2[s2][kj]
                                nc.tensor.matmul(
                                    po_ps[64 * s2:64 * s2 + 64,
                                          run[0] * PP:(run[-1] + 1) * PP],
                                    lhsT=vT_sb[:, kj, hh * 64:(hh + 1) * 64],
                                    rhs=pt[:, cols[run[0]]:
                                           cols[run[0]] + len(run) * PP],
                                    start=st0, stop=sp0)
                            i = j + 1
                    nc.vector.tensor_tensor(
                        o_sb[:, pair, :W0], po_ps[:, :W0],
                        rrs[:, :W0], OP.mult)

            # ---------------- patch embedding ----------------
            xf_sb = bigp.tile([PP, 24, S], bf16, tag="big")
            nc.sync.dma_start(
                out=xf_sb, in_=dp["xfT"][:].rearrange("(ko p) m -> p ko m", p=PP))
            pos_sb = act.tile([PP, 6, S], f32)
            nc.sync.dma_start(
                out=pos_sb, in_=dp["pos"][:].rearrange("(ko p) m -> p ko m", p=PP))
            h_sb = act.tile([PP, 6, S], f32)

            def embed_evac(m, ps):
                nc.vector.tensor_tensor(h_sb[:, m, :], ps, pos_sb[:, m, :],
                                        OP.add)
            dense("wk", 24, 6, xf_sb, S, embed_evac)

            if "dbg_h0" in dbg_ext:
                nc.sync.dma_start(out=dbg_ext["dbg_h0"][:], in_=h_sb)

            # ---------------- encoder layers ----------------
            y_sb = act.tile([PP, 6, S], bf16)
            q_sb = act.tile([PP, 6, S], bf16)
            k_sb = act.tile([PP, 6, S], bf16)
            vT_sb = act.tile([PP, 4, D], bf16)
            o_sb = act.tile([PP, 6, S], bf16)
            for l in range(L):
                lnorm(h_sb, 6, S, y_sb)
                qb = load_b(f"eqb{l}", 6)

                def qkv_evac(m, ps):
                    if m < 6:      # Q with bias
                        nc.vector.tensor_scalar_add(q_sb[:, m, :], ps,
                                                    qb[:, m:m + 1])
                    else:          # K plain
                        nc.vector.tensor_copy(k_sb[:, m - 6, :], ps)
                dense(f"eqk{l}", 6, 12, y_sb, S, qkv_evac)
                # V token-major: lhsT = y tile, rhs = Wv columns
                wv = load_w(f"evw{l}", 6, D)
                for jb in range(4):
                    for nch in range(2):
                        ncs = slice(nch * 384, (nch + 1) * 384)
                        ps = pp_mm.tile([PP, 512], f32, tag="mm")
                        for k in range(6):
                            nc.tensor.matmul(
                                ps[:, :384],
                                lhsT=y_sb[:, k, jb * PP:(jb + 1) * PP],
                                rhs=wv[:, k, ncs],
                                start=(k == 0), stop=(k == 5))
                        nc.vector.tensor_copy(vT_sb[:, jb, ncs], ps[:, :384])

                attention(q_sb, k_sb, vT_sb, o_sb, NH, 4, allowed[l])

                pb = load_b(f"epb{l}", 6)

                def proj_evac(m, ps):
                    t = tmp.tile([PP, S], f32, tag="ev")
                    nc.vector.tensor_scalar_add(t, ps, pb[:, m:m + 1])
                    nc.gpsimd.tensor_tensor(h_sb[:, m, :], h_sb[:, m, :], t,
                                            OP.add)
                dense(f"eproj{l}", 6, 6, o_sb, S, proj_evac)

                lnorm(h_sb, 6, S, y_sb)
                g_sb = bigp.tile([PP, 24, S], bf16, tag="big")
                m1b = load_b(f"em1b{l}", 24)

                def gelu_evac(m, ps):
                    nc.scalar.activation(g_sb[:, m, :], ps, AF.Gelu,
                                         bias=m1b[:, m:m + 1])
                dense(f"em1{l}", 6, 24, y_sb, S, gelu_evac)

                m2b = load_b(f"em2b{l}", 6)

                def mlp2_evac(m, ps):
                    t = tmp.tile([PP, S], f32, tag="ev")
                    nc.vector.tensor_scalar_add(t, ps, m2b[:, m:m + 1])
                    nc.gpsimd.tensor_tensor(h_sb[:, m, :], h_sb[:, m, :], t,
                                            OP.add)
                dense(f"em2{l}", 24, 6, g_sb, S, mlp2_evac)

                if f"dbg_he{l}" in dbg_ext:
                    nc.sync.dma_start(out=dbg_ext[f"dbg_he{l}"][:], in_=h_sb)

            # ---------------- encoder -> decoder ----------------
            nc.vector.tensor_copy(y_sb, h_sb)
            e2db = load_b("e2db", 4)
            memT_sb = act.tile([PP, 4, S], bf16)   # feature-major mem

            def e2d_evac(m, ps):
                nc.vector.tensor_scalar_add(memT_sb[:, m, :], ps,
                                            e2db[:, m:m + 1])
            dense("e2dw", 6, 4, y_sb, S, e2d_evac)

            # ---------------- decoder ----------------
            qd_sb = act.tile([PP, 4, P], f32)      # decoder residual stream
            nc.sync.dma_start(
                out=qd_sb, in_=dp["dq"][:].rearrange("(ko p) m -> p ko m", p=PP))

            yd_sb = act.tile([PP, 4, P], bf16)
            Qd_sb = act.tile([PP, 4, P], bf16)
            Kd_sb = act.tile([PP, 4, S], bf16)
            vTd_sb = act.tile([PP, 4, DD], bf16)
            od_sb = act.tile([PP, 4, P], bf16)
            gd_sb = act.tile([PP, 16, P], bf16)

            for l in range(LD):

                def kd_evac(m, ps):
                    nc.vector.tensor_copy(Kd_sb[:, m, :], ps)
                dense(f"dwk{l}", 4, 4, memT_sb, S, kd_evac)

                wvd = load_w(f"dwv{l}", 4, DD)
                for jb in range(4):
                    ps = pp_mm.tile([PP, 512], f32, tag="mm")
                    for k in range(4):
                        nc.tensor.matmul(
                            ps[:, :DD],
                            lhsT=memT_sb[:, k, jb * PP:(jb + 1) * PP],
                            rhs=wvd[:, k, :],
                            start=(k == 0), stop=(k == 3))
                    nc.vector.tensor_copy(vTd_sb[:, jb, :], ps[:, :DD])

                lnorm(qd_sb, 4, P, yd_sb)
                qbd = load_b(f"dqb{l}", 4)

                def qd_evac(m, ps):
                    nc.vector.tensor_scalar_add(Qd_sb[:, m, :], ps,
                                                qbd[:, m:m + 1])
                dense(f"dwq{l}", 4, 4, yd_sb, P, qd_evac)

                attention(Qd_sb, Kd_sb, vTd_sb, od_sb, NHD, 1,
                          [(0, 1, 2, 3)])

                obd = load_b(f"dob{l}", 4)

                def od_evac(m, ps):
                    t = tmp.tile([PP, S], f32, tag="ev")
                    nc.vector.tensor_scalar_add(t[:, :P], ps, obd[:, m:m + 1])
                    nc.gpsimd.tensor_tensor(qd_sb[:, m, :], qd_sb[:, m, :],
                                            t[:, :P], OP.add)
                dense(f"dwo{l}", 4, 4, od_sb, P, od_evac)

                lnorm(qd_sb, 4, P, yd_sb)
                m1bd = load_b(f"dm1b{l}", 16)

                def gelud_evac(m, ps):
                    nc.scalar.activation(gd_sb[:, m, :], ps, AF.Gelu,
                                         bias=m1bd[:, m:m + 1])
                dense(f"dm1{l}", 4, 16, yd_sb, P, gelud_evac)

                m2bd = load_b(f"dm2b{l}", 4)

                def mlp2d_evac(m, ps):
                    t = tmp.tile([PP, S], f32, tag="ev")
                    nc.vector.tensor_scalar_add(t[:, :P], ps, m2bd[:, m:m + 1])
                    nc.gpsimd.tensor_tensor(qd_sb[:, m, :], qd_sb[:, m, :],
                                            t[:, :P], OP.add)
                dense(f"dm2{l}", 16, 4, gd_sb, P, mlp2d_evac)

            # ---------------- head ----------------
            lnorm(qd_sb, 4, P, yd_sb)
            wh = load_w("hw", 4, OUT)
            hbb_sb = act.tile([PP, OUT], f32)
            nc.sync.dma_start(out=hbb_sb, in_=dp["hbb"][:])
            out_sb = act.tile([P, OUT], f32)
            for nchunk in range(2):
                ncs = slice(nchunk * 384, (nchunk + 1) * 384)
                ps = pp_mm.tile([PP, 512], f32, tag="mm")
                for k in range(4):
                    nc.tensor.matmul(ps[:, :384],
                                     lhsT=yd_sb[:, k, :],
                                     rhs=wh[:, k, ncs],
                                     start=(k == 0), stop=(k == 3))
                nc.vector.tensor_tensor(out_sb[:, ncs], ps[:, :384],
                                        hbb_sb[:, ncs], OP.add)
            nc.sync.dma_start(out=out_ext[:], in_=out_sb)

    return nc


# ------------------------------------------------------------------
# entry point
# ------------------------------------------------------------------

def kernel(dbg=(), _trace=False, _tmpdir=None, _full=False, **inputs):
    from concourse.bass_utils import run_bass_kernel_spmd

    w, xfT_bf, allowed = _prep(inputs)
    nc = _build(allowed, dbg=dbg)
    in_maps = []
    for b in range(B):
        m = dict(w)
        m["xfT"] = xfT_bf[b]
        in_maps.append(m)
    res = run_bass_kernel_spmd(nc, in_maps, core_ids=list(range(8)),
                               trace=_trace, tmpdir=_tmpdir)
    out = np.stack([np.asarray(res.results[i]["out"]) for i in range(B)])
    if dbg or _full:
        dbgs = {name: np.stack([np.asarray(res.results[i][name])
                                for i in range(B)]) for name in dbg}
        return out.astype(np.float32), dbgs, res
    return out.astype(np.float32)


# revision 27
# speedup vs baseline: 1.0326x; 1.0326x over previous
"""Trainium2 Bass kernel for nn_ARPredVideoVanilla (8-core data-parallel).

Strategy: pure data parallelism over batch (B=8 -> 1 batch element per core,
no collectives).  Activations live feature-major in SBUF: (128 partitions =
feature chunk, KO feature tiles, tokens free).  Weights are pre-folded on the
host (LN scale/bias folded into the following matmul, attention scale folded
into W_q, K-bias dropped (softmax row-shift invariant), V-bias folded into the
output-projection bias) and shipped as bf16.  The stochastic block mask is
evaluated on the host and compiled into the graph: blocked (query,key) frame
blocks are simply never computed.  Softmax runs without max-subtraction
(scores are O(1) by construction); exp runs on ScalarE with accum_out row sums.
"""

import sys

sys.path.insert(0, "/opt/trn_rl_repo")

import numpy as np
import ml_dtypes

BF16 = ml_dtypes.bfloat16

# ---- model dims (hardcoded from the problem spec) ----
B, T, V = 8, 4, 3
C = V * T                      # 12
H, W, PH, PW = 128, 256, 16, 16
HP, WP = H // PH, W // PW      # 8, 16
P = HP * WP                    # 128 patches/frame
S = T * P                      # 512
D, NH, HD, L = 768, 12, 64, 8
DD, NHD, HDD, LD = 512, 8, 64, 4
MLP, MLPD = 3072, 2048
OUT = PH * PW * V              # 768
MASK_RATIO = 0.8
EPS = 1e-5
PP = 128  # partitions


# ------------------------------------------------------------------
# host-side preparation: fold biases/scales, transpose, cast to bf16
# ------------------------------------------------------------------

def _prep(inputs):
    f32 = np.float32
    g = {k: np.asarray(v, dtype=f32) for k, v in inputs.items()}

    w = {}

    def bf(a):
        return np.ascontiguousarray(a.astype(BF16))

    def pcol(bias):  # (M,) -> (128, M//128) per-partition layout, tile-major
        M = bias.shape[0]
        return np.ascontiguousarray(bias.reshape(M // PP, PP).T.astype(f32))

    # patch data, per core: x[b] (T,C,H,W) -> xfT (C*PH*PW, T*P)
    x = g["x"]  # (B,T,C,H,W)
    xf = x.reshape(B, T, C, HP, PH, WP, PW).transpose(0, 1, 3, 5, 2, 4, 6)
    xf = xf.reshape(B, T * P, C * PH * PW)          # (B, 512, 3072)
    xfT = np.ascontiguousarray(np.swapaxes(xf, 1, 2))  # (B, 3072, 512)
    xfT_bf = [bf(xfT[b]) for b in range(B)]

    # conv: wk (3072, 768); pos_eff (768, 512) f32 with conv_b folded in
    wk = g["conv_w"].reshape(D, C * PH * PW).T      # (3072, 768)
    w["wk"] = bf(wk)
    pos = g["pos_emb"][0].T + g["conv_b"][:, None]  # (768, 512)
    w["pos"] = np.ascontiguousarray(pos.astype(f32))

    scale = HD ** -0.5
    for l in range(L):
        s1, b1 = g["enc_ln1_s"][l], g["enc_ln1_b"][l]
        Wqkv = g["enc_qkv_w"][l]                    # (768, 2304)
        Wq_eff = (s1[:, None] * Wqkv).copy()
        Wq_eff[:, :D] *= scale
        w[f"eqk{l}"] = bf(Wq_eff[:, :2 * D])
        w[f"evw{l}"] = bf(Wq_eff[:, 2 * D:])
        cb = b1 @ Wqkv                              # folded LN bias through qkv
        w[f"eqb{l}"] = pcol(cb[:D] * scale)         # q bias (per-partition)
        # k bias dropped (softmax row-invariant); v bias folded into proj bias
        w[f"eproj{l}"] = bf(g["enc_proj_w"][l])
        pb = g["enc_proj_b"][l] + cb[2 * D:] @ g["enc_proj_w"][l]
        w[f"epb{l}"] = pcol(pb)
        s2, b2 = g["enc_ln2_s"][l], g["enc_ln2_b"][l]
        W1 = g["enc_mlp_w1"][l]
        w[f"em1{l}"] = bf(s2[:, None] * W1)
        w[f"em1b{l}"] = pcol(b2 @ W1 + g["enc_mlp_b1"][l])
        w[f"em2{l}"] = bf(g["enc_mlp_w2"][l])
        w[f"em2b{l}"] = pcol(g["enc_mlp_b2"][l])

    w["e2dw"] = bf(g["e2d_w"])                      # (768, 512)
    w["e2db"] = pcol(g["e2d_b"])
    w["dq"] = np.ascontiguousarray(g["dec_query"][0].T.astype(f32))  # (512,128)

    dscale = HDD ** -0.5
    for l in range(LD):
        s1, b1 = g["dec_ln1_s"][l], g["dec_ln1_b"][l]
        Wq = g["dec_qkv_w"][l, 0]
        w[f"dwq{l}"] = bf(s1[:, None] * Wq * dscale)
        w[f"dqb{l}"] = pcol((b1 @ Wq + g["dec_qkv_b"][l, 0]) * dscale)
        w[f"dwk{l}"] = bf(g["dec_qkv_w"][l, 1])     # k bias dropped
        w[f"dwv{l}"] = bf(g["dec_qkv_w"][l, 2])
        w[f"dwo{l}"] = bf(g["dec_out_w"][l])
        ob = g["dec_out_b"][l] + g["dec_qkv_b"][l, 2] @ g["dec_out_w"][l]
        w[f"dob{l}"] = pcol(ob)
        s2, b2 = g["dec_ln2_s"][l], g["dec_ln2_b"][l]
        W1 = g["dec_mlp_w1"][l]
        w[f"dm1{l}"] = bf(s2[:, None] * W1)
        w[f"dm1b{l}"] = pcol(b2 @ W1 + g["dec_mlp_b1"][l])
        w[f"dm2{l}"] = bf(g["dec_mlp_w2"][l])
        w[f"dm2b{l}"] = pcol(g["dec_mlp_b2"][l])

    sh, bh = g["head_ln_s"], g["head_ln_b"]
    w["hw"] = bf(sh[:, None] * g["head_w"])         # (512, 768)
    hb = bh @ g["head_w"] + g["head_b"]             # (768,) per-FREE bias
    w["hbb"] = np.ascontiguousarray(
        np.broadcast_to(hb[None, :], (PP, OUT)).astype(f32))

    w["ident"] = np.ascontiguousarray(np.eye(PP, dtype=np.float32).astype(BF16))

    # block mask: allowed[l][qi] = tuple of allowed key-frame blocks
    mr = g["mask_rand"]                             # (L, T, T)
    allowed = []
    for l in range(L):
        per_q = []
        for i in range(T):
            ks = [j for j in range(T)
                  if j <= i or not (mr[l, i, j] < MASK_RATIO)]
            per_q.append(tuple(ks))
        allowed.append(per_q)

    return w, xfT_bf, allowed


# ------------------------------------------------------------------
# Tile tail-drain patch: this walrus build rejects >1 sync wait per
# instruction at the kernel-tail drain; split the waits across NOPs.
# ------------------------------------------------------------------

def _patch_tile():
    import concourse.tile as tile
    from concourse.vector_clock import ScopedClock, VectorClock

    if getattr(tile.TileContext, "_drain_patched", False):
        return

    def _drain_and_barrier_chunked(self, tick_clock, wait_clock):
        g = list(tick_clock.global_clock)
        procs = [i for i, v in enumerate(g) if v > 0]
        for p in procs:
            sub = [0] * len(g)
            sub[p] = g[p]
            nop_inst = self.nc.sync.nop(nofuse=True)
            wait_clock.add_sem_waits(
                nop_inst.ins, ScopedClock({None: VectorClock(sub)}))
        self.nc.sync.drain()
        self.nc.all_engine_barrier()
        assert self.sems is not None
        popped = self.nc._tile_sem_poison_stack.pop()
        assert popped is self._sem_poison
        self.nc.clear_and_free_semaphores(list(self.sems.allocated().values()))
        self.nc.all_engine_barrier()

    tile.TileContext._drain_and_barrier = _drain_and_barrier_chunked

    # This walrus build also rejects >1 sync wait on regular engine
    # instructions (Matmult etc.).  Hoist excess waits onto same-engine
    # NOPs inserted immediately before the instruction.
    from concourse import mybir as _mybir

    _orig_lower = tile.TileContext._lower_ordered_insts

    def _split_waits_and_lower(self, ordered):
        nctr = [0]
        for bb_name, insts in ordered.items():
            new_list = []
            for inst in insts:
                si = getattr(inst, "sync_info", None)
                waits = list(si.on_wait) if si is not None else []
                if len(waits) > 1:
                    imm = [w for w in waits if w.wait_reg is None]
                    reg = [w for w in waits if w.wait_reg is not None]
                    keep = imm[:1] + reg  # keep one imm (plus any reg waits)
                    excess = imm[1:]
                    for w in excess:
                        nctr[0] += 1
                        nop = _mybir.InstNoOp(
                            name=f"{inst.name}-wsplit{nctr[0]}", ins=[], outs=[])
                        nop.engine = inst.engine
                        nop.sync_info = _mybir.SyncInfo(
                            on_wait=[w], on_update=[])
                        self.nc.register_instruction(nop, overwrite=True)
                        new_list.append(nop)
                    inst.sync_info = _mybir.SyncInfo(
                        on_wait=keep, on_update=list(si.on_update))
                new_list.append(inst)
            insts[:] = new_list
        return _orig_lower(self, ordered)

    tile.TileContext._lower_ordered_insts = _split_waits_and_lower
    tile.TileContext._drain_patched = True


# ------------------------------------------------------------------
# graph builder
# ------------------------------------------------------------------

def _build(allowed, dbg=()):
    import concourse.bass as bass
    import concourse.tile as tile
    from concourse import mybir

    _patch_tile()
    f32 = mybir.dt.float32
    bf16 = mybir.dt.bfloat16
    AF = mybir.ActivationFunctionType
    OP = mybir.AluOpType

    nc = bass.Bass()

    # ---- DRAM parameters ----
    dp = {}

    def din(name, shape, dtype):
        dp[name] = nc.declare_dram_parameter(name, list(shape), dtype, isOutput=False)
        return dp[name]

    din("xfT", (24 * PP, S), bf16)
    din("wk", (24 * PP, D), bf16)
    din("pos", (D, S), f32)
    for l in range(L):
        din(f"eqk{l}", (D, 2 * D), bf16)
        din(f"evw{l}", (D, D), bf16)
        din(f"eqb{l}", (PP, 6), f32)
        din(f"eproj{l}", (D, D), bf16)
        din(f"epb{l}", (PP, 6), f32)
        din(f"em1{l}", (D, MLP), bf16)
        din(f"em1b{l}", (PP, 24), f32)
        din(f"em2{l}", (MLP, D), bf16)
        din(f"em2b{l}", (PP, 6), f32)
    din("e2dw", (D, DD), bf16)
    din("e2db", (PP, 4), f32)
    din("dq", (DD, P), f32)
    for l in range(LD):
        din(f"dwq{l}", (DD, DD), bf16)
        din(f"dqb{l}", (PP, 4), f32)
        din(f"dwk{l}", (DD, DD), bf16)
        din(f"dwv{l}", (DD, DD), bf16)
        din(f"dwo{l}", (DD, DD), bf16)
        din(f"dob{l}", (PP, 4), f32)
        din(f"dm1{l}", (DD, MLPD), bf16)
        din(f"dm1b{l}", (PP, 16), f32)
        din(f"dm2{l}", (MLPD, DD), bf16)
        din(f"dm2b{l}", (PP, 4), f32)
    din("hw", (DD, OUT), bf16)
    din("ident", (PP, PP), bf16)
    din("hbb", (PP, OUT), f32)
    out_ext = nc.declare_dram_parameter("out", [P, OUT], f32, isOutput=True)
    dbg_ext = {name: nc.declare_dram_parameter(name, [PP, 6, S], f32, isOutput=True)
               for name in dbg}

    with tile.TileContext(nc) as tc:
        with (
            tc.tile_pool(name="consts", bufs=1) as consts,
            tc.tile_pool(name="wpool", bufs=4) as wpool,
            tc.tile_pool(name="bias", bufs=6) as biasp,
            tc.tile_pool(name="act", bufs=1) as act,
            tc.tile_pool(name="tmp", bufs=2) as tmp,
            tc.tile_pool(name="hsqp", bufs=1) as hsqp,
            tc.tile_pool(name="bigp", bufs=1) as bigp,
            tc.tile_pool(name="attn", bufs=10) as attnp,
            tc.tile_pool(name="rrsp", bufs=2) as rrsp,
            tc.tile_pool(name="small", bufs=2) as small,
            tc.tile_pool(name="tiny", bufs=8) as tiny,
            tc.tile_pool(name="pp_mm", bufs=2, space="PSUM") as pp_mm,
            tc.tile_pool(name="pp_sc", bufs=2, space="PSUM") as pp_sc,
            tc.tile_pool(name="pp_pv", bufs=1, space="PSUM") as pp_pv,
            tc.tile_pool(name="pp_st", bufs=2, space="PSUM") as pp_st,
            tc.tile_pool(name="pp_bc", bufs=1, space="PSUM") as pp_bc,
        ):
            ones_f32 = consts.tile([PP, 1], f32)
            nc.vector.memset(ones_f32, 1.0)
            ones_row = consts.tile([1, PP], f32)
            nc.vector.memset(ones_row, 1.0)
            ones_bf16 = consts.tile([PP, 1], bf16)
            nc.vector.memset(ones_bf16, 1.0)
            ones_row_bf = consts.tile([1, PP], bf16)
            nc.vector.memset(ones_row_bf, 1.0)
            eps_t = consts.tile([1, 1], f32)
            nc.vector.memset(eps_t, EPS)

            def load_w(name, KO, M, dtype=bf16, tag="w"):
                t = wpool.tile([PP, KO, M], dtype, tag=tag)
                nc.sync.dma_start(
                    out=t, in_=dp[name][:].rearrange("(ko p) m -> p ko m", p=PP))
                return t

            W_SLOT = 6144  # bf16 elems per partition in a weight slot

            def load_b(name, MO):
                t = biasp.tile([PP, MO], f32, tag="b")
                nc.sync.dma_start(out=t, in_=dp[name][:])
                return t

            # dense matmul with chunked weight streaming from DRAM.
            # out feature-major; rhs (128, KO, N); evac(m, psum)
            def dense(wname, KO, MO, rhs_sb, N, evac):
                M = MO * PP
                mch_cols = max(PP, (W_SLOT // KO) // PP * PP)
                wap = dp[wname][:].rearrange("(ko p) m -> p ko m", p=PP)
                for c0 in range(0, M, mch_cols):
                    mch = min(mch_cols, M - c0)
                    wt = wpool.tile([PP, KO, mch], bf16, tag="w")
                    nc.sync.dma_start(out=wt, in_=wap[:, :, c0:c0 + mch])
                    for mi in range(mch // PP):
                        m = c0 // PP + mi
                        ps = pp_mm.tile([PP, 512], f32, tag="mm")
                        for k in range(KO):
                            nc.tensor.matmul(
                                ps[:, :N],
                                lhsT=wt[:, k, mi * PP:(mi + 1) * PP],
                                rhs=rhs_sb[:, k, :],
                                start=(k == 0), stop=(k == KO - 1))
                        evac(m, ps[:, :N])

            # layer norm, feature-major input (128, KO, W) f32 -> bf16 out
            def lnorm(h_sb, KO, Wd, y_sb):
                Dm = KO * PP
                hb = hsqp.tile([PP, KO, Wd], bf16, tag="hb")
                hsq = hsqp.tile([PP, KO, Wd], bf16, tag="hsq")
                st = pp_st.tile([33, Wd], f32, tag="st")
                for k in range(KO):
                    nc.vector.tensor_copy(hb[:, k, :], h_sb[:, k, :])
                    nc.tensor.matmul(st[0:1, :], lhsT=ones_bf16, rhs=hb[:, k, :],
                                     start=(k == 0), stop=(k == KO - 1))
                    nc.scalar.activation(hsq[:, k, :], h_sb[:, k, :], AF.Square)
                    nc.tensor.matmul(st[32:33, :], lhsT=ones_bf16, rhs=hsq[:, k, :],
                                     start=(k == 0), stop=(k == KO - 1))
                mean = small.tile([1, Wd], f32, tag="s1")
                nc.vector.tensor_scalar_mul(mean, st[0:1, :], 1.0 / Dm)
                var = small.tile([1, Wd], f32, tag="s2")
                nc.vector.tensor_scalar_mul(var, st[32:33, :], 1.0 / Dm)
                msq = small.tile([1, Wd], f32, tag="s3")
                nc.vector.tensor_mul(msq, mean, mean)
                nc.vector.tensor_sub(var, var, msq)
                # 1/sqrt(var+eps) = exp(-0.5*ln(var+eps)): stays in the
                # exp/ln ACT table set (no sqrt-set switch, no slow DVE
                # reciprocal)
                nc.scalar.activation(var, var, AF.Ln, bias=eps_t)
                inv = small.tile([1, Wd], f32, tag="s5")
                nc.scalar.activation(inv, var, AF.Exp, scale=-0.5)
                mean_bf = small.tile([1, Wd], bf16, tag="s6")
                nc.vector.tensor_copy(mean_bf, mean)
                inv_bf = small.tile([1, Wd], bf16, tag="s7")
                nc.vector.tensor_copy(inv_bf, inv)
                mb = pp_bc.tile([PP, Wd], f32, tag="bc")
                nc.tensor.matmul(mb, lhsT=ones_row_bf, rhs=mean_bf,
                                 start=True, stop=True)
                for k in range(KO):
                    nc.vector.tensor_tensor(
                        y_sb[:, k, :], h_sb[:, k, :], mb, OP.subtract)
                ib = pp_bc.tile([PP, Wd], f32, tag="bc")
                nc.tensor.matmul(ib, lhsT=ones_row_bf, rhs=inv_bf,
                                 start=True, stop=True)
                for k in range(KO):
                    nc.vector.tensor_tensor(
                        y_sb[:, k, :], y_sb[:, k, :], ib, OP.mult)

            # attention, transposed-scores formulation: no p transposes.
            # The two heads of a 128-partition pair are interleaved matmul-by-
            # matmul so they land on disjoint PE row/col groups and execute
            # concurrently.  Row sums for both heads share one PSUM tile at
            # partitions 0 and 32.
            def attention(q_sb, k_sb, vT_sb, o_sb, n_heads, n_q_tiles,
                          allowed_per_qi):
                kj_all = sorted({kj for qi in range(n_q_tiles)
                                 for kj in allowed_per_qi[qi]})
                kj_to_qi = {kj: [qi for qi in range(n_q_tiles)
                                 if kj in allowed_per_qi[qi]] for kj in kj_all}

                def qi_runs(qis):
                    runs = []
                    i = 0
                    while i < len(qis):
                        j = i
                        while j + 1 < len(qis) and qis[j + 1] == qis[j] + 1:
                            j += 1
                        runs.append(qis[i:j + 1])
                        i = j + 1
                    return runs

                W0 = n_q_tiles * PP
                # pack consecutive key blocks into <=512-col score tiles so
                # each ScalarE exp covers more columns (fixed ~352cyc op
                # overhead amortizes)
                if n_q_tiles == 1:
                    packs = []
                    cur = []
                    cur_cols = 0
                    for kj in kj_all:
                        nc_kj = len(kj_to_qi[kj]) * PP
                        if cur and cur_cols + nc_kj > 512:
                            packs.append(cur)
                            cur, cur_cols = [], 0
                        cur.append(kj)
                        cur_cols += nc_kj
                    if cur:
                        packs.append(cur)
                else:
                    packs = [[kj] for kj in kj_all]
                for pair in range(n_heads // 2):
                    pt2 = [{}, {}]
                    # scores + exp, subs interleaved per key block
                    for pack in packs:
                        pcols = sum(len(kj_to_qi[kj]) for kj in pack) * PP
                        sc2 = []
                        for sub in range(2):
                            sc2.append(pp_sc.tile([PP, 512], f32, tag="sc",
                                                  name=f"sc{sub}"))
                        base = 0
                        for kj in pack:
                            qis = kj_to_qi[kj]
                            for run in qi_runs(qis):
                                col = base + qis.index(run[0]) * PP
                                for sub in range(2):
                                    b0 = 64 * sub
                                    nc.tensor.matmul(
                                        sc2[sub][:, col:col + len(run) * PP],
                                        lhsT=k_sb[b0:b0 + 64, pair,
                                                  kj * PP:(kj + 1) * PP],
                                        rhs=q_sb[b0:b0 + 64, pair,
                                                 run[0] * PP:
                                                 (run[-1] + 1) * PP],
                                        start=True, stop=True)
                            base += len(qis) * PP
                        for sub in range(2):
                            pt = attnp.tile([PP, 512], bf16, tag="p")
                            nc.scalar.activation(pt[:, :pcols],
                                                 sc2[sub][:, :pcols], AF.Exp)
                            base = 0
                            for kj in pack:
                                qis = kj_to_qi[kj]
                                pt2[sub][kj] = (pt, {qi: base + i * PP
                                                     for i, qi
                                                     in enumerate(qis)})
                                base += len(qis) * PP
                    # row sums: separate tiles per sub (same-bank PE-write +
                    # ACT-read on disjoint partitions is a HW fault)
                    rsps2 = [pp_st.tile([1, 512], f32, tag="st", name="rs0"),
                             pp_st.tile([1, 512], f32, tag="st", name="rs1")]
                    seen = [[0] * n_q_tiles, [0] * n_q_tiles]
                    nkj = {qi: len(allowed_per_qi[qi])
                           for qi in range(n_q_tiles)}
                    for kj in kj_all:
                        qis = kj_to_qi[kj]
                        for run in qi_runs(qis):
                            for sub in range(2):
                                pt, cols = pt2[sub][kj]
                                nc.tensor.matmul(
                                    rsps2[sub][0:1,
                                               run[0] * PP:(run[-1] + 1) * PP],
                                    lhsT=ones_bf16,
                                    rhs=pt[:, cols[run[0]]:
                                           cols[run[0]] + len(run) * PP],
                                    start=(seen[sub][run[0]] == 0),
                                    stop=(seen[sub][run[0]]
                                          == nkj[run[0]] - 1))
                            for qi in run:
                                seen[0][qi] += 1
                                seen[1][qi] += 1
                    # 1/rowsum via exp(-ln(x)) on ScalarE, both subs
                    rr2 = []
                    for sub in range(2):
                        rr = small.tile([1, 512], f32, tag="rr", name=f"rrx{sub}")
                        lnr = small.tile([1, 512], f32, tag="s3")
                        nc.scalar.activation(lnr[:, :W0],
                                             rsps2[sub][:, :W0], AF.Ln)
                        nc.scalar.activation(rr[:, :W0], lnr[:, :W0],
                                             AF.Exp, scale=-1.0)
                        rr2.append(rr)
                    rrb = pp_bc.tile([PP, 512], f32, tag="bc")
                    nc.tensor.matmul(rrb[0:64, :W0], lhsT=ones_row[:, :64],
                                     rhs=rr2[0][:, :W0], start=True, stop=True)
                    nc.tensor.matmul(rrb[64:128, :W0], lhsT=ones_row[:, :64],
                                     rhs=rr2[1][:, :W0], start=True, stop=True)
                    rrs = rrsp.tile([PP, 512], bf16, tag="rrs")
                    nc.vector.tensor_copy(rrs[:, :W0], rrb[:, :W0])
                    # PV, subs interleaved, merged over contiguous qi runs
                    po_ps = pp_pv.tile([PP, 512], f32, tag="pv")
                    for kj in kj_all:
                        qis = kj_to_qi[kj]
                        i = 0
                        while i < len(qis):
                            qi0 = qis[i]
                            st0 = (kj == allowed_per_qi[qi0][0])
                            sp0 = (kj == allowed_per_qi[qi0][-1])
                            j = i
                            while (j + 1 < len(qis)
                                   and qis[j + 1] == qis[j] + 1
                                   and (kj == allowed_per_qi[
                                       qis[j + 1]][0]) == st0
                                   and (kj == allowed_per_qi[
                                       qis[j + 1]][-1]) == sp0):
                                j += 1
                            run = qis[i:j + 1]
                            for s2 in range(2):
                                hh = 2 * pair + s2
                                pt, cols = pt2[s2][kj]
                                nc.tensor.matmul(
                                    po_ps[64 * s2:64 * s2 + 64,
                                          run[0] * PP:(run[-1] + 1) * PP],
                                    lhsT=vT_sb[:, kj, hh * 64:(hh + 1) * 64],
                                    rhs=pt[:, cols[run[0]]:
                                           cols[run[0]] + len(run) * PP],
                                    start=st0, stop=sp0)
                            i = j + 1
                    nc.vector.tensor_tensor(
                        o_sb[:, pair, :W0], po_ps[:, :W0],
                        rrs[:, :W0], OP.mult)

            # ---------------- patch embedding ----------------
            xf_sb = bigp.tile([PP, 24, S], bf16, tag="big")
            nc.sync.dma_start(
                out=xf_sb, in_=dp["xfT"][:].rearrange("(ko p) m -> p ko m", p=PP))
            pos_sb = act.tile([PP, 6, S], f32)
            nc.sync.dma_start(
                out=pos_sb, in_=dp["pos"][:].rearrange("(ko p) m -> p ko m", p=PP))
            h_sb = act.tile([PP, 6, S], f32)

            def embed_evac(m, ps):
                nc.vector.tensor_tensor(h_sb[:, m, :], ps, pos_sb[:, m, :],
                                        OP.add)
            dense("wk", 24, 6, xf_sb, S, embed_evac)

            if "dbg_h0" in dbg_ext:
                nc.sync.dma_start(out=dbg_ext["dbg_h0"][:], in_=h_sb)

            # ---------------- encoder layers ----------------
            y_sb = act.tile([PP, 6, S], bf16)
            q_sb = act.tile([PP, 6, S], bf16)
            k_sb = act.tile([PP, 6, S], bf16)
            vT_sb = act.tile([PP, 4, D], bf16)
            o_sb = act.tile([PP, 6, S], bf16)
            for l in range(L):
                lnorm(h_sb, 6, S, y_sb)
                qb = load_b(f"eqb{l}", 6)

                def qkv_evac(m, ps):
                    if m < 6:      # Q with bias
                        nc.vector.tensor_scalar_add(q_sb[:, m, :], ps,
                                                    qb[:, m:m + 1])
                    else:          # K plain
                        nc.vector.tensor_copy(k_sb[:, m - 6, :], ps)
                dense(f"eqk{l}", 6, 12, y_sb, S, qkv_evac)
                # V token-major: lhsT = y tile, rhs = Wv columns
                wv = load_w(f"evw{l}", 6, D)
                for jb in range(4):
                    for nch in range(2):
                        ncs = slice(nch * 384, (nch + 1) * 384)
                        ps = pp_mm.tile([PP, 512], f32, tag="mm")
                        for k in range(6):
                            nc.tensor.matmul(
                                ps[:, :384],
                                lhsT=y_sb[:, k, jb * PP:(jb + 1) * PP],
                                rhs=wv[:, k, ncs],
                                start=(k == 0), stop=(k == 5))
                        nc.vector.tensor_copy(vT_sb[:, jb, ncs], ps[:, :384])

                attention(q_sb, k_sb, vT_sb, o_sb, NH, 4, allowed[l])

                pb = load_b(f"epb{l}", 6)

                def proj_evac(m, ps):
                    t = tmp.tile([PP, S], f32, tag="ev")
                    nc.vector.tensor_scalar_add(t, ps, pb[:, m:m + 1])
                    nc.gpsimd.tensor_tensor(h_sb[:, m, :], h_sb[:, m, :], t,
                                            OP.add)
                dense(f"eproj{l}", 6, 6, o_sb, S, proj_evac)

                lnorm(h_sb, 6, S, y_sb)
                g_sb = bigp.tile([PP, 24, S], bf16, tag="big")
                m1b = load_b(f"em1b{l}", 24)

                def gelu_evac(m, ps):
                    nc.scalar.activation(g_sb[:, m, :], ps, AF.Gelu,
                                         bias=m1b[:, m:m + 1])
                dense(f"em1{l}", 6, 24, y_sb, S, gelu_evac)

                m2b = load_b(f"em2b{l}", 6)

                def mlp2_evac(m, ps):
                    t = tmp.tile([PP, S], f32, tag="ev")
                    nc.vector.tensor_scalar_add(t, ps, m2b[:, m:m + 1])
                    nc.gpsimd.tensor_tensor(h_sb[:, m, :], h_sb[:, m, :], t,
                                            OP.add)
                dense(f"em2{l}", 24, 6, g_sb, S, mlp2_evac)

                if f"dbg_he{l}" in dbg_ext:
                    nc.sync.dma_start(out=dbg_ext[f"dbg_he{l}"][:], in_=h_sb)

            # ---------------- encoder -> decoder ----------------
            nc.vector.tensor_copy(y_sb, h_sb)
            e2db = load_b("e2db", 4)
            memT_sb = act.tile([PP, 4, S], bf16)   # feature-major mem

            def e2d_evac(m, ps):
                nc.vector.tensor_scalar_add(memT_sb[:, m, :], ps,
                                            e2db[:, m:m + 1])
            dense("e2dw", 6, 4, y_sb, S, e2d_evac)

            # ---------------- decoder ----------------
            qd_sb = act.tile([PP, 4, P], f32)      # decoder residual stream
            nc.sync.dma_start(
                out=qd_sb, in_=dp["dq"][:].rearrange("(ko p) m -> p ko m", p=PP))

            yd_sb = act.tile([PP, 4, P], bf16)
            Qd_sb = act.tile([PP, 4, P], bf16)
            Kd_sb = act.tile([PP, 4, S], bf16)
            vTd_sb = act.tile([PP, 4, DD], bf16)
            od_sb = act.tile([PP, 4, P], bf16)
            gd_sb = act.tile([PP, 16, P], bf16)

            for l in range(LD):

                def kd_evac(m, ps):
                    nc.vector.tensor_copy(Kd_sb[:, m, :], ps)
                dense(f"dwk{l}", 4, 4, memT_sb, S, kd_evac)

                wvd = load_w(f"dwv{l}", 4, DD)
                for jb in range(4):
                    ps = pp_mm.tile([PP, 512], f32, tag="mm")
                    for k in range(4):
                        nc.tensor.matmul(
                            ps[:, :DD],
                            lhsT=memT_sb[:, k, jb * PP:(jb + 1) * PP],
                            rhs=wvd[:, k, :],
                            start=(k == 0), stop=(k == 3))
                    nc.vector.tensor_copy(vTd_sb[:, jb, :], ps[:, :DD])

                lnorm(qd_sb, 4, P, yd_sb)
                qbd = load_b(f"dqb{l}", 4)

                def qd_evac(m, ps):
                    nc.vector.tensor_scalar_add(Qd_sb[:, m, :], ps,
                                                qbd[:, m:m + 1])
                dense(f"dwq{l}", 4, 4, yd_sb, P, qd_evac)

                attention(Qd_sb, Kd_sb, vTd_sb, od_sb, NHD, 1,
                          [(0, 1, 2, 3)])

                obd = load_b(f"dob{l}", 4)

                def od_evac(m, ps):
                    t = tmp.tile([PP, S], f32, tag="ev")
                    nc.vector.tensor_scalar_add(t[:, :P], ps, obd[:, m:m + 1])
                    nc.gpsimd.tensor_tensor(qd_sb[:, m, :], qd_sb[:, m, :],
                                            t[:, :P], OP.add)
                dense(f"dwo{l}", 4, 4, od_sb, P, od_evac)

                lnorm(qd_sb, 4, P, yd_sb)
                m1bd = load_b(f"dm1b{l}", 16)

                def gelud_evac(m, ps):
                    nc.scalar.activation(gd_sb[:, m, :], ps, AF.Gelu,
                                         bias=m1bd[:, m:m + 1])
                dense(f"dm1{l}", 4, 16, yd_sb, P, gelud_evac)

                m2bd = load_b(f"dm2b{l}", 4)

                def mlp2d_evac(m, ps):
                    t = tmp.tile([PP, S], f32, tag="ev")
                    nc.vector.tensor_scalar_add(t[:, :P], ps, m2bd[:, m:m + 1])
                    nc.gpsimd.tensor_tensor(qd_sb[:, m, :], qd_sb[:, m, :],
                                            t[:, :P], OP.add)
                dense(f"dm2{l}", 16, 4, gd_sb, P, mlp2d_evac)

            # ---------------- head ----------------
            lnorm(qd_sb, 4, P, yd_sb)
            wh = load_w("hw", 4, OUT)
            hbb_sb = act.tile([PP, OUT], f32)
            nc.sync.dma_start(out=hbb_sb, in_=dp["hbb"][:])
            out_sb = act.tile([P, OUT], f32)
            for nchunk in range(2):
                ncs = slice(nchunk * 384, (nchunk + 1) * 384)
                ps = pp_mm.tile([PP, 512], f32, tag="mm")
                for k in range(4):
                    nc.tensor.matmul(ps[:, :384],
                                     lhsT=yd_sb[:, k, :],
                                     rhs=wh[:, k, ncs],
                                     start=(k == 0), stop=(k == 3))
                nc.vector.tensor_tensor(out_sb[:, ncs], ps[:, :384],
                                        hbb_sb[:, ncs], OP.add)
            nc.sync.dma_start(out=out_ext[:], in_=out_sb)

    return nc


# ------------------------------------------------------------------
# entry point
# ------------------------------------------------------------------

def kernel(dbg=(), _trace=False, _tmpdir=None, _full=False, **inputs):
    from concourse.bass_utils import run_bass_kernel_spmd

    w, xfT_bf, allowed = _prep(inputs)
    nc = _build(allowed, dbg=dbg)
    in_maps = []
    for b in range(B):
        m = dict(w)
        m["xfT"] = xfT_bf[b]
        in_maps.append(m)
    res = run_bass_kernel_spmd(nc, in_maps, core_ids=list(range(8)),
                               trace=_trace, tmpdir=_tmpdir)
    out = np.stack([np.asarray(res.results[i]["out"]) for i in range(B)])
    if dbg or _full:
        dbgs = {name: np.stack([np.asarray(res.results[i][name])
                                for i in range(B)]) for name in dbg}
        return out.astype(np.float32), dbgs, res
    return out.astype(np.float32)


# revision 28
# speedup vs baseline: 1.0414x; 1.0085x over previous
"""Trainium2 Bass kernel for nn_ARPredVideoVanilla (8-core data-parallel).

Strategy: pure data parallelism over batch (B=8 -> 1 batch element per core,
no collectives).  Activations live feature-major in SBUF: (128 partitions =
feature chunk, KO feature tiles, tokens free).  Weights are pre-folded on the
host (LN scale/bias folded into the following matmul, attention scale folded
into W_q, K-bias dropped (softmax row-shift invariant), V-bias folded into the
output-projection bias) and shipped as bf16.  The stochastic block mask is
evaluated on the host and compiled into the graph: blocked (query,key) frame
blocks are simply never computed.  Softmax runs without max-subtraction
(scores are O(1) by construction); exp runs on ScalarE with accum_out row sums.
"""

import sys

sys.path.insert(0, "/opt/trn_rl_repo")

import numpy as np
import ml_dtypes

BF16 = ml_dtypes.bfloat16

# ---- model dims (hardcoded from the problem spec) ----
B, T, V = 8, 4, 3
C = V * T                      # 12
H, W, PH, PW = 128, 256, 16, 16
HP, WP = H // PH, W // PW      # 8, 16
P = HP * WP                    # 128 patches/frame
S = T * P                      # 512
D, NH, HD, L = 768, 12, 64, 8
DD, NHD, HDD, LD = 512, 8, 64, 4
MLP, MLPD = 3072, 2048
OUT = PH * PW * V              # 768
MASK_RATIO = 0.8
EPS = 1e-5
PP = 128  # partitions


# ------------------------------------------------------------------
# host-side preparation: fold biases/scales, transpose, cast to bf16
# ------------------------------------------------------------------

def _prep(inputs):
    f32 = np.float32
    g = {k: np.asarray(v, dtype=f32) for k, v in inputs.items()}

    w = {}

    def bf(a):
        return np.ascontiguousarray(a.astype(BF16))

    def pcol(bias):  # (M,) -> (128, M//128) per-partition layout, tile-major
        M = bias.shape[0]
        return np.ascontiguousarray(bias.reshape(M // PP, PP).T.astype(f32))

    # patch data, per core: x[b] (T,C,H,W) -> xfT (C*PH*PW, T*P)
    x = g["x"]  # (B,T,C,H,W)
    xf = x.reshape(B, T, C, HP, PH, WP, PW).transpose(0, 1, 3, 5, 2, 4, 6)
    xf = xf.reshape(B, T * P, C * PH * PW)          # (B, 512, 3072)
    xfT = np.ascontiguousarray(np.swapaxes(xf, 1, 2))  # (B, 3072, 512)
    xfT_bf = [bf(xfT[b]) for b in range(B)]

    # conv: wk (3072, 768); pos_eff (768, 512) f32 with conv_b folded in
    wk = g["conv_w"].reshape(D, C * PH * PW).T      # (3072, 768)
    w["wk"] = bf(wk)
    pos = g["pos_emb"][0].T + g["conv_b"][:, None]  # (768, 512)
    w["pos"] = np.ascontiguousarray(pos.astype(f32))

    scale = HD ** -0.5
    for l in range(L):
        s1, b1 = g["enc_ln1_s"][l], g["enc_ln1_b"][l]
        Wqkv = g["enc_qkv_w"][l]                    # (768, 2304)
        Wq_eff = (s1[:, None] * Wqkv).copy()
        Wq_eff[:, :D] *= scale
        w[f"eqk{l}"] = bf(Wq_eff[:, :2 * D])
        w[f"evw{l}"] = bf(Wq_eff[:, 2 * D:])
        cb = b1 @ Wqkv                              # folded LN bias through qkv
        w[f"eqb{l}"] = pcol(cb[:D] * scale)         # q bias (per-partition)
        # k bias dropped (softmax row-invariant); v bias folded into proj bias
        w[f"eproj{l}"] = bf(g["enc_proj_w"][l])
        pb = g["enc_proj_b"][l] + cb[2 * D:] @ g["enc_proj_w"][l]
        w[f"epb{l}"] = pcol(pb)
        s2, b2 = g["enc_ln2_s"][l], g["enc_ln2_b"][l]
        W1 = g["enc_mlp_w1"][l]
        w[f"em1{l}"] = bf(s2[:, None] * W1)
        w[f"em1b{l}"] = pcol(b2 @ W1 + g["enc_mlp_b1"][l])
        w[f"em2{l}"] = bf(g["enc_mlp_w2"][l])
        w[f"em2b{l}"] = pcol(g["enc_mlp_b2"][l])

    w["e2dw"] = bf(g["e2d_w"])                      # (768, 512)
    w["e2db"] = pcol(g["e2d_b"])
    w["dq"] = np.ascontiguousarray(g["dec_query"][0].T.astype(f32))  # (512,128)

    dscale = HDD ** -0.5
    for l in range(LD):
        s1, b1 = g["dec_ln1_s"][l], g["dec_ln1_b"][l]
        Wq = g["dec_qkv_w"][l, 0]
        w[f"dwq{l}"] = bf(s1[:, None] * Wq * dscale)
        w[f"dqb{l}"] = pcol((b1 @ Wq + g["dec_qkv_b"][l, 0]) * dscale)
        w[f"dwk{l}"] = bf(g["dec_qkv_w"][l, 1])     # k bias dropped
        w[f"dwv{l}"] = bf(g["dec_qkv_w"][l, 2])
        w[f"dwo{l}"] = bf(g["dec_out_w"][l])
        ob = g["dec_out_b"][l] + g["dec_qkv_b"][l, 2] @ g["dec_out_w"][l]
        w[f"dob{l}"] = pcol(ob)
        s2, b2 = g["dec_ln2_s"][l], g["dec_ln2_b"][l]
        W1 = g["dec_mlp_w1"][l]
        w[f"dm1{l}"] = bf(s2[:, None] * W1)
        w[f"dm1b{l}"] = pcol(b2 @ W1 + g["dec_mlp_b1"][l])
        w[f"dm2{l}"] = bf(g["dec_mlp_w2"][l])
        w[f"dm2b{l}"] = pcol(g["dec_mlp_b2"][l])

    sh, bh = g["head_ln_s"], g["head_ln_b"]
    w["hw"] = bf(sh[:, None] * g["head_w"])         # (512, 768)
    hb = bh @ g["head_w"] + g["head_b"]             # (768,) per-FREE bias
    w["hbb"] = np.ascontiguousarray(
        np.broadcast_to(hb[None, :], (PP, OUT)).astype(f32))

    w["ident"] = np.ascontiguousarray(np.eye(PP, dtype=np.float32).astype(BF16))

    # block mask: allowed[l][qi] = tuple of allowed key-frame blocks
    mr = g["mask_rand"]                             # (L, T, T)
    allowed = []
    for l in range(L):
        per_q = []
        for i in range(T):
            ks = [j for j in range(T)
                  if j <= i or not (mr[l, i, j] < MASK_RATIO)]
            per_q.append(tuple(ks))
        allowed.append(per_q)

    return w, xfT_bf, allowed


# ------------------------------------------------------------------
# Tile tail-drain patch: this walrus build rejects >1 sync wait per
# instruction at the kernel-tail drain; split the waits across NOPs.
# ------------------------------------------------------------------

def _patch_tile():
    import concourse.tile as tile
    from concourse.vector_clock import ScopedClock, VectorClock

    if getattr(tile.TileContext, "_drain_patched", False):
        return

    def _drain_and_barrier_chunked(self, tick_clock, wait_clock):
        g = list(tick_clock.global_clock)
        procs = [i for i, v in enumerate(g) if v > 0]
        for p in procs:
            sub = [0] * len(g)
            sub[p] = g[p]
            nop_inst = self.nc.sync.nop(nofuse=True)
            wait_clock.add_sem_waits(
                nop_inst.ins, ScopedClock({None: VectorClock(sub)}))
        self.nc.sync.drain()
        self.nc.all_engine_barrier()
        assert self.sems is not None
        popped = self.nc._tile_sem_poison_stack.pop()
        assert popped is self._sem_poison
        self.nc.clear_and_free_semaphores(list(self.sems.allocated().values()))
        self.nc.all_engine_barrier()

    tile.TileContext._drain_and_barrier = _drain_and_barrier_chunked

    # This walrus build also rejects >1 sync wait on regular engine
    # instructions (Matmult etc.).  Hoist excess waits onto same-engine
    # NOPs inserted immediately before the instruction.
    from concourse import mybir as _mybir

    _orig_lower = tile.TileContext._lower_ordered_insts

    def _split_waits_and_lower(self, ordered):
        nctr = [0]
        for bb_name, insts in ordered.items():
            new_list = []
            for inst in insts:
                si = getattr(inst, "sync_info", None)
                waits = list(si.on_wait) if si is not None else []
                if len(waits) > 1:
                    imm = [w for w in waits if w.wait_reg is None]
                    reg = [w for w in waits if w.wait_reg is not None]
                    keep = imm[:1] + reg  # keep one imm (plus any reg waits)
                    excess = imm[1:]
                    for w in excess:
                        nctr[0] += 1
                        nop = _mybir.InstNoOp(
                            name=f"{inst.name}-wsplit{nctr[0]}", ins=[], outs=[])
                        nop.engine = inst.engine
                        nop.sync_info = _mybir.SyncInfo(
                            on_wait=[w], on_update=[])
                        self.nc.register_instruction(nop, overwrite=True)
                        new_list.append(nop)
                    inst.sync_info = _mybir.SyncInfo(
                        on_wait=keep, on_update=list(si.on_update))
                new_list.append(inst)
            insts[:] = new_list
        return _orig_lower(self, ordered)

    tile.TileContext._lower_ordered_insts = _split_waits_and_lower
    tile.TileContext._drain_patched = True


# ------------------------------------------------------------------
# graph builder
# ------------------------------------------------------------------

def _build(allowed, dbg=()):
    import concourse.bass as bass
    import concourse.tile as tile
    from concourse import mybir

    _patch_tile()
    f32 = mybir.dt.float32
    bf16 = mybir.dt.bfloat16
    AF = mybir.ActivationFunctionType
    OP = mybir.AluOpType

    nc = bass.Bass()

    # ---- DRAM parameters ----
    dp = {}

    def din(name, shape, dtype):
        dp[name] = nc.declare_dram_parameter(name, list(shape), dtype, isOutput=False)
        return dp[name]

    din("xfT", (24 * PP, S), bf16)
    din("wk", (24 * PP, D), bf16)
    din("pos", (D, S), f32)
    for l in range(L):
        din(f"eqk{l}", (D, 2 * D), bf16)
        din(f"evw{l}", (D, D), bf16)
        din(f"eqb{l}", (PP, 6), f32)
        din(f"eproj{l}", (D, D), bf16)
        din(f"epb{l}", (PP, 6), f32)
        din(f"em1{l}", (D, MLP), bf16)
        din(f"em1b{l}", (PP, 24), f32)
        din(f"em2{l}", (MLP, D), bf16)
        din(f"em2b{l}", (PP, 6), f32)
    din("e2dw", (D, DD), bf16)
    din("e2db", (PP, 4), f32)
    din("dq", (DD, P), f32)
    for l in range(LD):
        din(f"dwq{l}", (DD, DD), bf16)
        din(f"dqb{l}", (PP, 4), f32)
        din(f"dwk{l}", (DD, DD), bf16)
        din(f"dwv{l}", (DD, DD), bf16)
        din(f"dwo{l}", (DD, DD), bf16)
        din(f"dob{l}", (PP, 4), f32)
        din(f"dm1{l}", (DD, MLPD), bf16)
        din(f"dm1b{l}", (PP, 16), f32)
        din(f"dm2{l}", (MLPD, DD), bf16)
        din(f"dm2b{l}", (PP, 4), f32)
    din("hw", (DD, OUT), bf16)
    din("ident", (PP, PP), bf16)
    din("hbb", (PP, OUT), f32)
    out_ext = nc.declare_dram_parameter("out", [P, OUT], f32, isOutput=True)
    dbg_ext = {name: nc.declare_dram_parameter(name, [PP, 6, S], f32, isOutput=True)
               for name in dbg}

    with tile.TileContext(nc) as tc:
        with (
            tc.tile_pool(name="consts", bufs=1) as consts,
            tc.tile_pool(name="wpool", bufs=4) as wpool,
            tc.tile_pool(name="bias", bufs=6) as biasp,
            tc.tile_pool(name="act", bufs=1) as act,
            tc.tile_pool(name="tmp", bufs=2) as tmp,
            tc.tile_pool(name="hsqp", bufs=1) as hsqp,
            tc.tile_pool(name="bigp", bufs=1) as bigp,
            tc.tile_pool(name="attn", bufs=10) as attnp,
            tc.tile_pool(name="rrsp", bufs=2) as rrsp,
            tc.tile_pool(name="small", bufs=2) as small,
            tc.tile_pool(name="tiny", bufs=8) as tiny,
            tc.tile_pool(name="pp_mm", bufs=3, space="PSUM") as pp_mm,
            tc.tile_pool(name="pp_sc", bufs=2, space="PSUM") as pp_sc,
            tc.tile_pool(name="pp_pv", bufs=1, space="PSUM") as pp_pv,
            tc.tile_pool(name="pp_st", bufs=1, space="PSUM") as pp_st,
            tc.tile_pool(name="pp_bc", bufs=1, space="PSUM") as pp_bc,
        ):
            ones_f32 = consts.tile([PP, 1], f32)
            nc.vector.memset(ones_f32, 1.0)
            ones_row = consts.tile([1, PP], f32)
            nc.vector.memset(ones_row, 1.0)
            ones_bf16 = consts.tile([PP, 1], bf16)
            nc.vector.memset(ones_bf16, 1.0)
            ones_row_bf = consts.tile([1, PP], bf16)
            nc.vector.memset(ones_row_bf, 1.0)
            eps_t = consts.tile([1, 1], f32)
            nc.vector.memset(eps_t, EPS)

            def load_w(name, KO, M, dtype=bf16, tag="w"):
                t = wpool.tile([PP, KO, M], dtype, tag=tag)
                nc.sync.dma_start(
                    out=t, in_=dp[name][:].rearrange("(ko p) m -> p ko m", p=PP))
                return t

            W_SLOT = 6144  # bf16 elems per partition in a weight slot

            def load_b(name, MO):
                t = biasp.tile([PP, MO], f32, tag="b")
                nc.sync.dma_start(out=t, in_=dp[name][:])
                return t

            # dense matmul with chunked weight streaming from DRAM.
            # out feature-major; rhs (128, KO, N); evac(m, psum)
            def dense(wname, KO, MO, rhs_sb, N, evac):
                M = MO * PP
                mch_cols = max(PP, (W_SLOT // KO) // PP * PP)
                wap = dp[wname][:].rearrange("(ko p) m -> p ko m", p=PP)
                for c0 in range(0, M, mch_cols):
                    mch = min(mch_cols, M - c0)
                    wt = wpool.tile([PP, KO, mch], bf16, tag="w")
                    nc.sync.dma_start(out=wt, in_=wap[:, :, c0:c0 + mch])
                    for mi in range(mch // PP):
                        m = c0 // PP + mi
                        ps = pp_mm.tile([PP, 512], f32, tag="mm")
                        for k in range(KO):
                            nc.tensor.matmul(
                                ps[:, :N],
                                lhsT=wt[:, k, mi * PP:(mi + 1) * PP],
                                rhs=rhs_sb[:, k, :],
                                start=(k == 0), stop=(k == KO - 1))
                        evac(m, ps[:, :N])

            # layer norm, feature-major input (128, KO, W) f32 -> bf16 out
            def lnorm(h_sb, KO, Wd, y_sb):
                Dm = KO * PP
                hb = hsqp.tile([PP, KO, Wd], bf16, tag="hb")
                hsq = hsqp.tile([PP, KO, Wd], bf16, tag="hsq")
                st = pp_st.tile([33, Wd], f32, tag="st")
                for k in range(KO):
                    nc.vector.tensor_copy(hb[:, k, :], h_sb[:, k, :])
                    nc.tensor.matmul(st[0:1, :], lhsT=ones_bf16, rhs=hb[:, k, :],
                                     start=(k == 0), stop=(k == KO - 1))
                    nc.scalar.activation(hsq[:, k, :], h_sb[:, k, :], AF.Square)
                    nc.tensor.matmul(st[32:33, :], lhsT=ones_bf16, rhs=hsq[:, k, :],
                                     start=(k == 0), stop=(k == KO - 1))
                mean = small.tile([1, Wd], f32, tag="s1")
                nc.vector.tensor_scalar_mul(mean, st[0:1, :], 1.0 / Dm)
                var = small.tile([1, Wd], f32, tag="s2")
                nc.vector.tensor_scalar_mul(var, st[32:33, :], 1.0 / Dm)
                msq = small.tile([1, Wd], f32, tag="s3")
                nc.vector.tensor_mul(msq, mean, mean)
                nc.vector.tensor_sub(var, var, msq)
                # 1/sqrt(var+eps) = exp(-0.5*ln(var+eps)): stays in the
                # exp/ln ACT table set (no sqrt-set switch, no slow DVE
                # reciprocal)
                nc.scalar.activation(var, var, AF.Ln, bias=eps_t)
                inv = small.tile([1, Wd], f32, tag="s5")
                nc.scalar.activation(inv, var, AF.Exp, scale=-0.5)
                mean_bf = small.tile([1, Wd], bf16, tag="s6")
                nc.vector.tensor_copy(mean_bf, mean)
                inv_bf = small.tile([1, Wd], bf16, tag="s7")
                nc.vector.tensor_copy(inv_bf, inv)
                mb = pp_bc.tile([PP, Wd], f32, tag="bc")
                nc.tensor.matmul(mb, lhsT=ones_row_bf, rhs=mean_bf,
                                 start=True, stop=True)
                for k in range(KO):
                    nc.vector.tensor_tensor(
                        y_sb[:, k, :], h_sb[:, k, :], mb, OP.subtract)
                ib = pp_bc.tile([PP, Wd], f32, tag="bc")
                nc.tensor.matmul(ib, lhsT=ones_row_bf, rhs=inv_bf,
                                 start=True, stop=True)
                for k in range(KO):
                    nc.vector.tensor_tensor(
                        y_sb[:, k, :], y_sb[:, k, :], ib, OP.mult)

            # attention, transposed-scores formulation: no p transposes.
            # The two heads of a 128-partition pair are interleaved matmul-by-
            # matmul so they land on disjoint PE row/col groups and execute
            # concurrently.  Row sums for both heads share one PSUM tile at
            # partitions 0 and 32.
            def attention(q_sb, k_sb, vT_sb, o_sb, n_heads, n_q_tiles,
                          allowed_per_qi):
                kj_all = sorted({kj for qi in range(n_q_tiles)
                                 for kj in allowed_per_qi[qi]})
                kj_to_qi = {kj: [qi for qi in range(n_q_tiles)
                                 if kj in allowed_per_qi[qi]] for kj in kj_all}

                def qi_runs(qis):
                    runs = []
                    i = 0
                    while i < len(qis):
                        j = i
                        while j + 1 < len(qis) and qis[j + 1] == qis[j] + 1:
                            j += 1
                        runs.append(qis[i:j + 1])
                        i = j + 1
                    return runs

                W0 = n_q_tiles * PP
                # pack consecutive key blocks into <=512-col score tiles so
                # each ScalarE exp covers more columns (fixed ~352cyc op
                # overhead amortizes)
                if n_q_tiles == 1:
                    packs = []
                    cur = []
                    cur_cols = 0
                    for kj in kj_all:
                        nc_kj = len(kj_to_qi[kj]) * PP
                        if cur and cur_cols + nc_kj > 512:
                            packs.append(cur)
                            cur, cur_cols = [], 0
                        cur.append(kj)
                        cur_cols += nc_kj
                    if cur:
                        packs.append(cur)
                else:
                    packs = [[kj] for kj in kj_all]
                for pair in range(n_heads // 2):
                    pt2 = [{}, {}]
                    # scores + exp, subs interleaved per key block
                    for pack in packs:
                        pcols = sum(len(kj_to_qi[kj]) for kj in pack) * PP
                        sc2 = []
                        for sub in range(2):
                            sc2.append(pp_sc.tile([PP, 512], f32, tag="sc",
                                                  name=f"sc{sub}"))
                        base = 0
                        for kj in pack:
                            qis = kj_to_qi[kj]
                            for run in qi_runs(qis):
                                col = base + qis.index(run[0]) * PP
                                for sub in range(2):
                                    b0 = 64 * sub
                                    nc.tensor.matmul(
                                        sc2[sub][:, col:col + len(run) * PP],
                                        lhsT=k_sb[b0:b0 + 64, pair,
                                                  kj * PP:(kj + 1) * PP],
                                        rhs=q_sb[b0:b0 + 64, pair,
                                                 run[0] * PP:
                                                 (run[-1] + 1) * PP],
                                        start=True, stop=True)
                            base += len(qis) * PP
                        for sub in range(2):
                            pt = attnp.tile([PP, 512], bf16, tag="p")
                            nc.scalar.activation(pt[:, :pcols],
                                                 sc2[sub][:, :pcols], AF.Exp)
                            base = 0
                            for kj in pack:
                                qis = kj_to_qi[kj]
                                pt2[sub][kj] = (pt, {qi: base + i * PP
                                                     for i, qi
                                                     in enumerate(qis)})
                                base += len(qis) * PP
                    # row sums: separate tiles per sub (same-bank PE-write +
                    # ACT-read on disjoint partitions is a HW fault)
                    rsps2 = [pp_st.tile([1, 512], f32, tag="st", name="rs0"),
                             pp_st.tile([1, 512], f32, tag="st", name="rs1")]
                    seen = [[0] * n_q_tiles, [0] * n_q_tiles]
                    nkj = {qi: len(allowed_per_qi[qi])
                           for qi in range(n_q_tiles)}
                    for kj in kj_all:
                        qis = kj_to_qi[kj]
                        for run in qi_runs(qis):
                            for sub in range(2):
                                pt, cols = pt2[sub][kj]
                                nc.tensor.matmul(
                                    rsps2[sub][0:1,
                                               run[0] * PP:(run[-1] + 1) * PP],
                                    lhsT=ones_bf16,
                                    rhs=pt[:, cols[run[0]]:
                                           cols[run[0]] + len(run) * PP],
                                    start=(seen[sub][run[0]] == 0),
                                    stop=(seen[sub][run[0]]
                                          == nkj[run[0]] - 1))
                            for qi in run:
                                seen[0][qi] += 1
                                seen[1][qi] += 1
                    # 1/rowsum via exp(-ln(x)) on ScalarE, both subs
                    rr2 = []
                    for sub in range(2):
                        rr = small.tile([1, 512], f32, tag="rr", name=f"rrx{sub}")
                        lnr = small.tile([1, 512], f32, tag="s3")
                        nc.scalar.activation(lnr[:, :W0],
                                             rsps2[sub][:, :W0], AF.Ln)
                        nc.scalar.activation(rr[:, :W0], lnr[:, :W0],
                                             AF.Exp, scale=-1.0)
                        rr2.append(rr)
                    rrb = pp_bc.tile([PP, 512], f32, tag="bc")
                    nc.tensor.matmul(rrb[0:64, :W0], lhsT=ones_row[:, :64],
                                     rhs=rr2[0][:, :W0], start=True, stop=True)
                    nc.tensor.matmul(rrb[64:128, :W0], lhsT=ones_row[:, :64],
                                     rhs=rr2[1][:, :W0], start=True, stop=True)
                    rrs = rrsp.tile([PP, 512], bf16, tag="rrs")
                    nc.vector.tensor_copy(rrs[:, :W0], rrb[:, :W0])
                    # PV, subs interleaved, merged over contiguous qi runs
                    po_ps = pp_pv.tile([PP, 512], f32, tag="pv")
                    for kj in kj_all:
                        qis = kj_to_qi[kj]
                        i = 0
                        while i < len(qis):
                            qi0 = qis[i]
                            st0 = (kj == allowed_per_qi[qi0][0])
                            sp0 = (kj == allowed_per_qi[qi0][-1])
                            j = i
                            while (j + 1 < len(qis)
                                   and qis[j + 1] == qis[j] + 1
                                   and (kj == allowed_per_qi[
                                       qis[j + 1]][0]) == st0
                                   and (kj == allowed_per_qi[
                                       qis[j + 1]][-1]) == sp0):
                                j += 1
                            run = qis[i:j + 1]
                            for s2 in range(2):
                                hh = 2 * pair + s2
                                pt, cols = pt2[s2][kj]
                                nc.tensor.matmul(
                                    po_ps[64 * s2:64 * s2 + 64,
                                          run[0] * PP:(run[-1] + 1) * PP],
                                    lhsT=vT_sb[:, kj, hh * 64:(hh + 1) * 64],
                                    rhs=pt[:, cols[run[0]]:
                                           cols[run[0]] + len(run) * PP],
                                    start=st0, stop=sp0)
                            i = j + 1
                    nc.vector.tensor_tensor(
                        o_sb[:, pair, :W0], po_ps[:, :W0],
                        rrs[:, :W0], OP.mult)

            # ---------------- patch embedding ----------------
            xf_sb = bigp.tile([PP, 24, S], bf16, tag="big")
            nc.sync.dma_start(
                out=xf_sb, in_=dp["xfT"][:].rearrange("(ko p) m -> p ko m", p=PP))
            pos_sb = act.tile([PP, 6, S], f32)
            nc.sync.dma_start(
                out=pos_sb, in_=dp["pos"][:].rearrange("(ko p) m -> p ko m", p=PP))
            h_sb = act.tile([PP, 6, S], f32)

            def embed_evac(m, ps):
                nc.vector.tensor_tensor(h_sb[:, m, :], ps, pos_sb[:, m, :],
                                        OP.add)
            dense("wk", 24, 6, xf_sb, S, embed_evac)

            if "dbg_h0" in dbg_ext:
                nc.sync.dma_start(out=dbg_ext["dbg_h0"][:], in_=h_sb)

            # ---------------- encoder layers ----------------
            y_sb = act.tile([PP, 6, S], bf16)
            q_sb = act.tile([PP, 6, S], bf16)
            k_sb = act.tile([PP, 6, S], bf16)
            vT_sb = act.tile([PP, 4, D], bf16)
            o_sb = act.tile([PP, 6, S], bf16)
            for l in range(L):
                lnorm(h_sb, 6, S, y_sb)
                qb = load_b(f"eqb{l}", 6)

                def qkv_evac(m, ps):
                    if m < 6:      # Q with bias
                        nc.vector.tensor_scalar_add(q_sb[:, m, :], ps,
                                                    qb[:, m:m + 1])
                    else:          # K plain
                        nc.vector.tensor_copy(k_sb[:, m - 6, :], ps)
                dense(f"eqk{l}", 6, 12, y_sb, S, qkv_evac)
                # V token-major: lhsT = y tile, rhs = Wv columns
                wv = load_w(f"evw{l}", 6, D)
                for jb in range(4):
                    for nch in range(2):
                        ncs = slice(nch * 384, (nch + 1) * 384)
                        ps = pp_mm.tile([PP, 512], f32, tag="mm")
                        for k in range(6):
                            nc.tensor.matmul(
                                ps[:, :384],
                                lhsT=y_sb[:, k, jb * PP:(jb + 1) * PP],
                                rhs=wv[:, k, ncs],
                                start=(k == 0), stop=(k == 5))
                        nc.vector.tensor_copy(vT_sb[:, jb, ncs], ps[:, :384])

                attention(q_sb, k_sb, vT_sb, o_sb, NH, 4, allowed[l])

                pb = load_b(f"epb{l}", 6)

                def proj_evac(m, ps):
                    t = tmp.tile([PP, S], f32, tag="ev")
                    nc.vector.tensor_scalar_add(t, ps, pb[:, m:m + 1])
                    nc.gpsimd.tensor_tensor(h_sb[:, m, :], h_sb[:, m, :], t,
                                            OP.add)
                dense(f"eproj{l}", 6, 6, o_sb, S, proj_evac)

                lnorm(h_sb, 6, S, y_sb)
                g_sb = bigp.tile([PP, 24, S], bf16, tag="big")
                m1b = load_b(f"em1b{l}", 24)

                def gelu_evac(m, ps):
                    nc.scalar.activation(g_sb[:, m, :], ps, AF.Gelu,
                                         bias=m1b[:, m:m + 1])
                dense(f"em1{l}", 6, 24, y_sb, S, gelu_evac)

                m2b = load_b(f"em2b{l}", 6)

                def mlp2_evac(m, ps):
                    t = tmp.tile([PP, S], f32, tag="ev")
                    nc.vector.tensor_scalar_add(t, ps, m2b[:, m:m + 1])
                    nc.gpsimd.tensor_tensor(h_sb[:, m, :], h_sb[:, m, :], t,
                                            OP.add)
                dense(f"em2{l}", 24, 6, g_sb, S, mlp2_evac)

                if f"dbg_he{l}" in dbg_ext:
                    nc.sync.dma_start(out=dbg_ext[f"dbg_he{l}"][:], in_=h_sb)

            # ---------------- encoder -> decoder ----------------
            nc.vector.tensor_copy(y_sb, h_sb)
            e2db = load_b("e2db", 4)
            memT_sb = act.tile([PP, 4, S], bf16)   # feature-major mem

            def e2d_evac(m, ps):
                nc.vector.tensor_scalar_add(memT_sb[:, m, :], ps,
                                            e2db[:, m:m + 1])
            dense("e2dw", 6, 4, y_sb, S, e2d_evac)

            # ---------------- decoder ----------------
            qd_sb = act.tile([PP, 4, P], f32)      # decoder residual stream
            nc.sync.dma_start(
                out=qd_sb, in_=dp["dq"][:].rearrange("(ko p) m -> p ko m", p=PP))

            yd_sb = act.tile([PP, 4, P], bf16)
            Qd_sb = act.tile([PP, 4, P], bf16)
            Kd_sb = act.tile([PP, 4, S], bf16)
            vTd_sb = act.tile([PP, 4, DD], bf16)
            od_sb = act.tile([PP, 4, P], bf16)
            gd_sb = act.tile([PP, 16, P], bf16)

            for l in range(LD):

                def kd_evac(m, ps):
                    nc.vector.tensor_copy(Kd_sb[:, m, :], ps)
                dense(f"dwk{l}", 4, 4, memT_sb, S, kd_evac)

                wvd = load_w(f"dwv{l}", 4, DD)
                for jb in range(4):
                    ps = pp_mm.tile([PP, 512], f32, tag="mm")
                    for k in range(4):
                        nc.tensor.matmul(
                            ps[:, :DD],
                            lhsT=memT_sb[:, k, jb * PP:(jb + 1) * PP],
                            rhs=wvd[:, k, :],
                            start=(k == 0), stop=(k == 3))
                    nc.vector.tensor_copy(vTd_sb[:, jb, :], ps[:, :DD])

                lnorm(qd_sb, 4, P, yd_sb)
                qbd = load_b(f"dqb{l}", 4)

                def qd_evac(m, ps):
                    nc.vector.tensor_scalar_add(Qd_sb[:, m, :], ps,
                                                qbd[:, m:m + 1])
                dense(f"dwq{l}", 4, 4, yd_sb, P, qd_evac)

                attention(Qd_sb, Kd_sb, vTd_sb, od_sb, NHD, 1,
                          [(0, 1, 2, 3)])

                obd = load_b(f"dob{l}", 4)

                def od_evac(m, ps):
                    t = tmp.tile([PP, S], f32, tag="ev")
                    nc.vector.tensor_scalar_add(t[:, :P], ps, obd[:, m:m + 1])
                    nc.gpsimd.tensor_tensor(qd_sb[:, m, :], qd_sb[:, m, :],
                                            t[:, :P], OP.add)
                dense(f"dwo{l}", 4, 4, od_sb, P, od_evac)

                lnorm(qd_sb, 4, P, yd_sb)
                m1bd = load_b(f"dm1b{l}", 16)

                def gelud_evac(m, ps):
                    nc.scalar.activation(gd_sb[:, m, :], ps, AF.Gelu,
                                         bias=m1bd[:, m:m + 1])
                dense(f"dm1{l}", 4, 16, yd_sb, P, gelud_evac)

                m2bd = load_b(f"dm2b{l}", 4)

                def mlp2d_evac(m, ps):
                    t = tmp.tile([PP, S], f32, tag="ev")
                    nc.vector.tensor_scalar_add(t[:, :P], ps, m2bd[:, m:m + 1])
                    nc.gpsimd.tensor_tensor(qd_sb[:, m, :], qd_sb[:, m, :],
                                            t[:, :P], OP.add)
                dense(f"dm2{l}", 16, 4, gd_sb, P, mlp2d_evac)

            # ---------------- head ----------------
            lnorm(qd_sb, 4, P, yd_sb)
            wh = load_w("hw", 4, OUT)
            hbb_sb = act.tile([PP, OUT], f32)
            nc.sync.dma_start(out=hbb_sb, in_=dp["hbb"][:])
            out_sb = act.tile([P, OUT], f32)
            for nchunk in range(2):
                ncs = slice(nchunk * 384, (nchunk + 1) * 384)
                ps = pp_mm.tile([PP, 512], f32, tag="mm")
                for k in range(4):
                    nc.tensor.matmul(ps[:, :384],
                                     lhsT=yd_sb[:, k, :],
                                     rhs=wh[:, k, ncs],
                                     start=(k == 0), stop=(k == 3))
                nc.vector.tensor_tensor(out_sb[:, ncs], ps[:, :384],
                                        hbb_sb[:, ncs], OP.add)
            nc.sync.dma_start(out=out_ext[:], in_=out_sb)

    return nc


# ------------------------------------------------------------------
# entry point
# ------------------------------------------------------------------

def kernel(dbg=(), _trace=False, _tmpdir=None, _full=False, **inputs):
    from concourse.bass_utils import run_bass_kernel_spmd

    w, xfT_bf, allowed = _prep(inputs)
    nc = _build(allowed, dbg=dbg)
    in_maps = []
    for b in range(B):
        m = dict(w)
        m["xfT"] = xfT_bf[b]
        in_maps.append(m)
    res = run_bass_kernel_spmd(nc, in_maps, core_ids=list(range(8)),
                               trace=_trace, tmpdir=_tmpdir)
    out = np.stack([np.asarray(res.results[i]["out"]) for i in range(B)])
    if dbg or _full:
        dbgs = {name: np.stack([np.asarray(res.results[i][name])
                                for i in range(B)]) for name in dbg}
        return out.astype(np.float32), dbgs, res
    return out.astype(np.float32)
